# revision 1
# baseline (speedup 1.0000x reference)
"""MetaPathGNN forward on 8 Trainium2 NeuronCores (Bass/Tile).

Strategy (self-contained; shapes hardcoded for N=100000, C=256, OUT=128, E=400000):
  - Nodes sharded 12500/core. Per layer: each core computes hw = h @ wlT for its
    nodes (fp16), AllGather -> full [100000,256] fp16 message table in DRAM.
  - Edges assigned to cores by src owner; host sorts each core's edges by
    (super-tile(src), int16-window(dst), tile(src), src) and pads so the slot
    layout is identical across cores (single SPMD NEFF).
  - Messages gathered with GpSimd dma_gather (512B fp16 rows, int16 indices
    against 25000-row windows of the table).
  - Segment-sum = fp8(0/1 selector) x fp16(messages) matmuls accumulated in
    PSUM, one [128,256] region per 128-node tile.
  - Epilogue fuses deg-normalize (x inv_deg per partition) + dense term + bias,
    relu + LN stats on ScalarE, normalize to fp16.
  - Dense terms computed as fp16 matmuls (node-major, lhsT = transposed
    activations; x provided pre-transposed by host, h transposed on PE).
"""
import os
import numpy as np
from contextlib import ExitStack

N = 100000
C = 256
OUT = 128
NCORES = 8
NPC = N // NCORES          # 12500 nodes per core
P = 128
TILES = (NPC + P - 1) // P  # 98
NPC_PAD = TILES * P         # 12544
ST_TILES = 8                # node-tiles per super-tile
NST = (TILES + ST_TILES - 1) // ST_TILES  # 13
WIN = 25000                 # window stride; indices in [0, 32768)
NWIN = 4
LN_EPS = 1e-5

_COMPILED = {}


# ---------------------------------------------------------------- host side
def _sigmoid(x):
    return 1.0 / (1.0 + np.exp(-np.float64(x)))


def _build_layer(src, dst):
    """Vectorized layout builder. Returns dict with:
       structure: per st -> list of (w, n_slots, mm_list(block,tile), slot_base, mm_base)
       idx:  [NCORES, S] int16 window-local gather indices
       selv: [NCORES, NMM*128] uint8 col (src col within tile) or 255 for none
             -> converted to fp8 selector blob [NCORES, 128, NMM*128]
       counts etc.
    """
    per_core = []
    diag = np.zeros((NCORES, 128, TILES * 128), dtype=np.uint8)
    for c in range(NCORES):
        lo = c * NPC
        m = (src >= lo) & (src < lo + NPC)
        # structural self-edges bypass the gather: counted into a diagonal blob
        selfm = m & (src == dst)
        si = (src[selfm] - lo).astype(np.int64)
        mult = np.bincount(si, minlength=NPC_PAD)
        pp = np.arange(NPC_PAD)
        diag[c, pp & 127, (pp >> 7) * 128 + (pp & 127)] = mult
        m = m & (src != dst)
        s = (src[m] - lo).astype(np.int64)
        d = dst[m].astype(np.int64)
        w = d // WIN
        t = s >> 7
        sti = t >> 3
        order = np.lexsort((s, t, w, sti))
        per_core.append((s[order], d[order], w[order], t[order], sti[order]))

    cnt = np.zeros((NCORES, NST, NWIN, TILES), dtype=np.int64)
    for c in range(NCORES):
        s, d, w, t, sti = per_core[c]
        np.add.at(cnt[c], (sti, w, t), 1)
    ucnt = cnt.max(axis=0)

    structure = []
    total_slots = 0
    total_mms = 0
    # structural slot->tile map and slot->mm map, plus per-(st,w,tile) unified slot starts
    slot_tile_all = []
    seg_start = {}  # (st,w,tile) -> unified slot start position (within global slot stream)
    for sti in range(NST):
        st_runs = []
        for wi in range(NWIN):
            segs = [(ti, int(ucnt[sti, wi, ti]))
                    for ti in range(sti * ST_TILES, min((sti + 1) * ST_TILES, TILES))
                    if ucnt[sti, wi, ti] > 0]
            slots = []
            tiles_in_blk = set()
            f = 0
            for (ti, n) in segs:
                if f > 0 and len(tiles_in_blk) >= 2 and ti not in tiles_in_blk:
                    slots.extend([-1] * (128 - f))
                    f = 0
                    tiles_in_blk = set()
                seg_start[(sti, wi, ti)] = total_slots + len(slots)
                rem = n
                while rem > 0:
                    take = min(128 - f, rem)
                    slots.extend([ti] * take)
                    f += take
                    rem -= take
                    tiles_in_blk.add(ti)
                    if f == 128:
                        f = 0
                        tiles_in_blk = set()
            if f > 0:
                slots.extend([-1] * (128 - f))
            n_slots = len(slots)
            nblk = n_slots // 128
            mm_list = []
            for b in range(nblk):
                blk = slots[b * 128:(b + 1) * 128]
                touched = []
                for q in blk:
                    if q >= 0 and q not in touched:
                        touched.append(q)
                for ti in touched:
                    mm_list.append((b, ti))
            st_runs.append({"w": wi, "n_slots": n_slots, "mm_list": mm_list,
                            "slot_base": total_slots, "mm_base": total_mms})
            slot_tile_all.extend(slots)
            total_slots += n_slots
            total_mms += len(mm_list)
        structure.append(st_runs)

    slot_tile_all = np.asarray(slot_tile_all, dtype=np.int64)

    # per-core: place edges into unified slots
    idx = np.zeros((NCORES, total_slots), dtype=np.int16)
    selcol = np.full((NCORES, total_slots), -1, dtype=np.int64)  # src col (s%128)
    for c in range(NCORES):
        s, d, w, t, sti = per_core[c]
        # edges sorted by (st, w, tile, s) -> contiguous per (st,w,tile)
        # compute per-edge offset within its (st,w,tile) group
        key = (sti * NWIN + w) * TILES + t
        # group offsets
        change = np.empty(len(key), dtype=bool)
        if len(key):
            change[0] = True
            change[1:] = key[1:] != key[:-1]
        grp_start_idx = np.flatnonzero(change)
        grp_of_edge = np.cumsum(change) - 1
        offset_in_grp = np.arange(len(key)) - grp_start_idx[grp_of_edge]
        base = np.array([seg_start[(int(sti[i]), int(w[i]), int(t[i]))]
                         for i in grp_start_idx], dtype=np.int64)
        slot_pos = base[grp_of_edge] + offset_in_grp
        idx[c, slot_pos] = (d - w * WIN).astype(np.int16)
        selcol[c, slot_pos] = s & 127

    # selector blob: [128 partitions(slot within block), total_mms*128] values 0/1
    # entry: for mm (b, ti) of run: slot j in block b with slot_tile==ti and
    # per-core selcol >= 0 -> sel[j, mm*128 + selcol] = 1
    f8 = np.float32  # staged as f32, cast at the end
    sel = np.zeros((NCORES, 128, total_mms * 128), dtype=np.uint8)
    mm_i_global = 0
    for sti in range(NST):
        for run in structure[sti]:
            sb = run["slot_base"]
            for (b, ti) in run["mm_list"]:
                sl0 = sb + b * 128
                tile_match = slot_tile_all[sl0:sl0 + 128] == ti
                for c in range(NCORES):
                    cols = selcol[c, sl0:sl0 + 128]
                    jj = np.flatnonzero(tile_match & (cols >= 0))
                    sel[c, jj, mm_i_global * 128 + cols[jj]] = 1
                mm_i_global += 1
    assert mm_i_global == total_mms

    # wrapped idx arrays: [128, total_slots/16]; slot i -> [i%16 (+16g), i//16]
    assert total_slots % 128 == 0
    idx_w = np.zeros((NCORES, 16, total_slots // 16), dtype=np.int16)
    ar = np.arange(total_slots)
    idx_w[:, ar % 16, ar // 16] = idx
    idx_w = np.tile(idx_w, (1, 8, 1))

    return {"structure": structure, "total_slots": total_slots, "total_mms": total_mms,
            "idx": idx_w, "sel": sel, "diag": diag}


def _prep(inputs):
    """All host-side preprocessing -> per-core input maps + static meta."""
    import ml_dtypes
    f16 = np.float16
    x = np.asarray(inputs["x"], np.float32)
    ei1 = np.asarray(inputs["edge_index_r1"])
    ei0 = np.asarray(inputs["edge_index_r0"])

    g1 = np.float32(_sigmoid(inputs["gate1"]))
    g0 = np.float32(_sigmoid(inputs["gate0"]))
    lns1 = np.asarray(inputs["lns1"], np.float32); lnb1 = np.asarray(inputs["lnb1"], np.float32)
    lns0 = np.asarray(inputs["lns0"], np.float32); lnb0 = np.asarray(inputs["lnb0"], np.float32)

    wl1 = np.asarray(inputs["wl1"], np.float32); bl1 = np.asarray(inputs["bl1"], np.float32)
    w01 = np.asarray(inputs["w01"], np.float32); b01 = np.asarray(inputs["b01"], np.float32)
    w11 = np.asarray(inputs["w11"], np.float32); b11 = np.asarray(inputs["b11"], np.float32)
    wl0 = np.asarray(inputs["wl0"], np.float32); bl0 = np.asarray(inputs["bl0"], np.float32)
    w00 = np.asarray(inputs["w00"], np.float32); b00 = np.asarray(inputs["b00"], np.float32)
    w10 = np.asarray(inputs["w10"], np.float32); b10 = np.asarray(inputs["b10"], np.float32)
    Wout = np.asarray(inputs["Wout"], np.float32); bout = np.asarray(inputs["bout"], np.float32)

    # Layer 1 (uses r1 edges, params *1); h = x0 = x
    rhsAB1 = np.concatenate([wl1.T, ((1 - g1) * w01 + g1 * w11).T], axis=1)  # [256,512]
    bias1 = bl1 + (1 - g1) * b01 + g1 * b11                                   # [256]
    # Layer 2 (r0 edges, params *0); h = h1 = h1r*lns1 + lnb1, x0 = x
    wl0T_s = (lns1[:, None] * wl0.T)            # [256,256] for h1r path
    const1 = lnb1 @ wl0.T                        # [256] message const
    w00T_s = (1 - g0) * (lns1[:, None] * w00.T)
    rhsA2 = np.concatenate([wl0T_s, w00T_s], axis=1)  # [256,512]
    w10T_s = g0 * w10.T                          # [256,256] x path
    bias2 = bl0 + (1 - g0) * (b00 + lnb1 @ w00.T) + g0 * b10
    crow2 = np.concatenate([const1, bias2])      # [512]
    # Final: out = h2 @ WoutT + bout, h2 = h2r*lns0 + lnb0
    WoutT_s = lns0[:, None] * Wout.T             # [256,128]
    bout_s = bout + lnb0 @ Wout.T                # [128]

    # degree normalization (bincount over dst of the FULL edge list)
    inv1 = 1.0 / np.clip(np.bincount(ei1[1], minlength=N), 1.0, None).astype(np.float32)
    inv0 = 1.0 / np.clip(np.bincount(ei0[1], minlength=N), 1.0, None).astype(np.float32)

    lay1 = _build_layer(ei1[0].astype(np.int64), ei1[1].astype(np.int64))
    lay2 = _build_layer(ei0[0].astype(np.int64), ei0[1].astype(np.int64))

    fp8 = ml_dtypes.float8_e4m3
    in_maps = []
    for c in range(NCORES):
        lo = c * NPC
        xs = np.zeros((NPC_PAD, C), np.float32)
        xs[:NPC] = x[lo:lo + NPC]
        inv1c = np.zeros(NPC_PAD, np.float32); inv1c[:NPC] = inv1[lo:lo + NPC]
        inv0c = np.zeros(NPC_PAD, np.float32); inv0c[:NPC] = inv0[lo:lo + NPC]
        in_maps.append(dict(
            xT=np.ascontiguousarray(xs.T).astype(f16),
            rhsAB1=rhsAB1.astype(f16), brow1=bias1[None, :].astype(f16),
            rhsA2=rhsA2.astype(f16), rhsY2=w10T_s.astype(f16), crow2=crow2[None, :].astype(f16),
            rhsF=WoutT_s.astype(f16), browF=bout_s[None, :].astype(f16),
            invdeg1=np.ascontiguousarray(inv1c.reshape(TILES, P).T),
            invdeg2=np.ascontiguousarray(inv0c.reshape(TILES, P).T),
            idx1=lay1["idx"][c], idx2=lay2["idx"][c],
            sel1=lay1["sel"][c].astype(np.float32).astype(fp8),
            sel2=lay2["sel"][c].astype(np.float32).astype(fp8),
            diag1=lay1["diag"][c].astype(np.float32).astype(fp8),
            diag2=lay2["diag"][c].astype(np.float32).astype(fp8),
        ))
    return in_maps, lay1, lay2


# ---------------------------------------------------------------- device side
def _build_nc(lay1, lay2):
    import concourse.bass as bass
    import concourse.tile as tile
    from concourse import bacc, mybir
    from concourse.masks import make_identity

    f32, f16 = mybir.dt.float32, mybir.dt.float16
    f8, i16 = mybir.dt.float8e4, mybir.dt.int16
    AF = mybir.ActivationFunctionType
    OP = mybir.AluOpType

    nc = bacc.Bacc("TRN2", target_bir_lowering=False, debug=False, num_devices=NCORES)

    S1, S2 = lay1["total_slots"], lay2["total_slots"]
    M1, M2 = lay1["total_mms"], lay2["total_mms"]

    xT_in = nc.dram_tensor("xT", [C, NPC_PAD], f16, kind="ExternalInput").ap()
    rhsAB1_in = nc.dram_tensor("rhsAB1", [C, 512], f16, kind="ExternalInput").ap()
    brow1_in = nc.dram_tensor("brow1", [1, 256], f16, kind="ExternalInput").ap()
    rhsA2_in = nc.dram_tensor("rhsA2", [C, 512], f16, kind="ExternalInput").ap()
    rhsY2_in = nc.dram_tensor("rhsY2", [C, 256], f16, kind="ExternalInput").ap()
    crow2_in = nc.dram_tensor("crow2", [1, 512], f16, kind="ExternalInput").ap()
    rhsF_in = nc.dram_tensor("rhsF", [C, OUT], f16, kind="ExternalInput").ap()
    browF_in = nc.dram_tensor("browF", [1, OUT], f16, kind="ExternalInput").ap()
    invdeg1_in = nc.dram_tensor("invdeg1", [P, TILES], f32, kind="ExternalInput").ap()
    invdeg2_in = nc.dram_tensor("invdeg2", [P, TILES], f32, kind="ExternalInput").ap()
    idx1_in = nc.dram_tensor("idx1", [P, S1 // 16], i16, kind="ExternalInput").ap()
    idx2_in = nc.dram_tensor("idx2", [P, S2 // 16], i16, kind="ExternalInput").ap()
    sel1_in = nc.dram_tensor("sel1", [P, M1 * P], f8, kind="ExternalInput").ap()
    sel2_in = nc.dram_tensor("sel2", [P, M2 * P], f8, kind="ExternalInput").ap()
    diag1_in = nc.dram_tensor("diag1", [P, TILES * P], f8, kind="ExternalInput").ap()
    diag2_in = nc.dram_tensor("diag2", [P, TILES * P], f8, kind="ExternalInput").ap()

    out_dram = nc.dram_tensor("out", [NPC_PAD, OUT], f32, kind="ExternalOutput").ap()
    DBG = os.environ.get("K_DEBUG", "0") == "1"
    if DBG:
        dbg_z1 = nc.dram_tensor("dbg_z1", [P, TILES * C], f16, kind="ExternalOutput").ap()
        dbg_pre1 = nc.dram_tensor("dbg_pre1", [P, TILES * C], f32, kind="ExternalOutput").ap()
        dbg_h1 = nc.dram_tensor("dbg_h1", [P, TILES * C], f16, kind="ExternalOutput").ap()
        dbg_hw1 = nc.dram_tensor("dbg_hw1", [P, TILES * C], f16, kind="ExternalOutput").ap()
        dbg_agg1 = nc.dram_tensor("dbg_agg1", [P, TILES * C], f32, kind="ExternalOutput").ap()
        dbg_g1 = nc.dram_tensor("dbg_g1", [P, 64 * C], f16, kind="ExternalOutput").ap()

    ag_in = nc.dram_tensor("ag_in", [NPC, C], f16)
    ag_in2 = nc.dram_tensor("ag_in2", [NPC, C], f16)
    table1 = nc.dram_tensor("table1", [N, C], f16, addr_space="Shared")
    table2 = nc.dram_tensor("table2", [N, C], f16, addr_space="Shared")

    max_blk_per_st = 0
    max_mm_per_st = 0
    for lay in (lay1, lay2):
        for st_runs in lay["structure"]:
            nb = sum(r["n_slots"] for r in st_runs) // 128
            nm = sum(len(r["mm_list"]) for r in st_runs)
            max_blk_per_st = max(max_blk_per_st, nb)
            max_mm_per_st = max(max_mm_per_st, nm)

    with tile.TileContext(nc) as tc, ExitStack() as ctx:
        sb = ctx.enter_context(tc.tile_pool(name="sb", bufs=1))
        lhs_pool = ctx.enter_context(tc.tile_pool(name="lhs", bufs=4))
        small = ctx.enter_context(tc.tile_pool(name="small", bufs=2))
        gpool = ctx.enter_context(tc.tile_pool(name="gst", bufs=2))
        spool = ctx.enter_context(tc.tile_pool(name="sel", bufs=2))
        epi = ctx.enter_context(tc.tile_pool(name="epi", bufs=4))

        # persistent staging
        z_sb = sb.tile([P, TILES * C], f16)      # dense term, then relu output r
        h_sb = sb.tile([P, TILES * C], f16)      # LN output (h1r, then h2r)
        s1_all = sb.tile([P, TILES], f32)
        s2_all = sb.tile([P, TILES], f32)
        mu_all = sb.tile([P, TILES], f32)
        rstd_all = sb.tile([P, TILES], f32)
        invdeg1_sb = sb.tile([P, TILES], f32)
        invdeg2_sb = sb.tile([P, TILES], f32)
        ident16 = sb.tile([P, P], f16)
        make_identity(nc, ident16[:])
        ones_col = sb.tile([1, P], f16)
        nc.vector.memset(ones_col[:], 1.0)
        nc.sync.dma_start(invdeg1_sb[:], invdeg1_in[:])
        nc.sync.dma_start(invdeg2_sb[:], invdeg2_in[:])

        # weight tiles (persistent)
        rhsAB1_sb = sb.tile([C // 2, 2, 512], f16)
        nc.sync.dma_start(rhsAB1_sb[:], rhsAB1_in[:].rearrange("(b k) n -> k b n", k=128))
        brow1_sb = sb.tile([1, 256], f16)
        nc.sync.dma_start(brow1_sb[:], brow1_in[:])
        rhsA2_sb = sb.tile([C // 2, 2, 512], f16)
        nc.sync.dma_start(rhsA2_sb[:], rhsA2_in[:].rearrange("(b k) n -> k b n", k=128))
        rhsY2_sb = sb.tile([C // 2, 2, 256], f16)
        nc.sync.dma_start(rhsY2_sb[:], rhsY2_in[:].rearrange("(b k) n -> k b n", k=128))
        crow2_sb = sb.tile([1, 512], f16)
        nc.sync.dma_start(crow2_sb[:], crow2_in[:])
        rhsF_sb = sb.tile([C // 2, 2, OUT], f16)
        nc.sync.dma_start(rhsF_sb[:], rhsF_in[:].rearrange("(b k) n -> k b n", k=128))
        browF_sb = sb.tile([1, OUT], f16)
        nc.sync.dma_start(browF_sb[:], browF_in[:])

        idx1_sb = sb.tile([P, S1 // 16], i16)
        nc.sync.dma_start(idx1_sb[:], idx1_in[:])
        idx2_sb = sb.tile([P, S2 // 16], i16)
        nc.sync.dma_start(idx2_sb[:], idx2_in[:])

        def dense_phase(layer):
            """Phase AB: compute [hw | z] per tile; hw -> ag bounce, z -> z_sb."""
            with tc.tile_pool(name="abps", bufs=4, space="PSUM") as abps, \
                 tc.tile_pool(name="trps", bufs=4, space="PSUM") as trps, \
                 tc.tile_pool(name="abe", bufs=4) as abe:
                for t in range(TILES):
                    acc = abps.tile([P, 512], f32, space="PSUM")
                    if layer == 1:
                        # lhsT = xT tile (DMA per K-block), rhs = rhsAB1
                        lh = lhs_pool.tile([P, 2, P], f16, tag="xTt")
                        nc.sync.dma_start(lh[:], xT_in[:, t * P:(t + 1) * P].rearrange("(b k) n -> k b n", k=128))
                        nc.tensor.matmul(acc[:], lhsT=lh[:, 0, :], rhs=rhsAB1_sb[:, 0, :], start=True, stop=False)
                        nc.tensor.matmul(acc[:], lhsT=lh[:, 1, :], rhs=rhsAB1_sb[:, 1, :], start=False, stop=False)
                        nc.tensor.matmul(acc[:, 256:512], lhsT=ones_col[:], rhs=brow1_sb[:],
                                         start=False, stop=True, skip_group_check=True)
                    else:
                        # transpose h1r tile -> lhsT fp16
                        hT = abe.tile([P, 2, P], f16, tag="hT")
                        for k in range(2):
                            tp = trps.tile([P, P], f16, space="PSUM", tag="trp")
                            nc.tensor.transpose(tp[:], h_sb[:, t * C + k * P: t * C + (k + 1) * P], ident16[:])
                            nc.vector.tensor_copy(hT[:, k, :], tp[:])
                        lh = lhs_pool.tile([P, 2, P], f16, tag="xTt")
                        nc.sync.dma_start(lh[:], xT_in[:, t * P:(t + 1) * P].rearrange("(b k) n -> k b n", k=128))
                        nc.tensor.matmul(acc[:], lhsT=hT[:, 0, :], rhs=rhsA2_sb[:, 0, :], start=True, stop=False)
                        nc.tensor.matmul(acc[:], lhsT=hT[:, 1, :], rhs=rhsA2_sb[:, 1, :], start=False, stop=False)
                        nc.tensor.matmul(acc[:, 256:512], lhsT=lh[:, 0, :], rhs=rhsY2_sb[:, 0, :],
                                         start=False, stop=False, skip_group_check=True)
                        nc.tensor.matmul(acc[:, 256:512], lhsT=lh[:, 1, :], rhs=rhsY2_sb[:, 1, :],
                                         start=False, stop=False, skip_group_check=True)
                        nc.tensor.matmul(acc[:], lhsT=ones_col[:], rhs=crow2_sb[:],
                                         start=False, stop=True, skip_group_check=True)
                    # hw half -> fp16 -> bounce DRAM
                    hw16 = abe.tile([P, C], f16, tag="hw16")
                    nc.scalar.activation(hw16[:], acc[:, 0:256], AF.Copy)
                    rows = min(P, NPC - t * P)
                    if rows > 0:
                        dstb = ag_in if layer == 1 else ag_in2
                        nc.sync.dma_start(dstb[t * P: t * P + rows, :], hw16[:rows, :])
                    if DBG and layer == 1:
                        nc.sync.dma_start(dbg_hw1[:, t * C:(t + 1) * C], hw16[:])
                    # z half -> z_sb fp16
                    nc.vector.tensor_copy(z_sb[:, t * C:(t + 1) * C], acc[:, 256:512])

        def gather_phase(layer, lay, table, idx_sb, sel_in_ap, invdeg_sb, diag_in_ap, agin):
            structure = lay["structure"]
            with tc.tile_pool(name="cps", bufs=8, space="PSUM") as cps:
                for sti in range(NST):
                    st_runs = structure[sti]
                    st_nblk = sum(r["n_slots"] for r in st_runs) // 128
                    st_nmm = sum(len(r["mm_list"]) for r in st_runs)
                    st_t0 = sti * ST_TILES
                    st_ntiles = min(ST_TILES, TILES - st_t0)
                    if st_nblk == 0:
                        continue
                    g_sb = gpool.tile([P, max_blk_per_st * C], f16, tag="g")
                    sel_sb = spool.tile([P, max_mm_per_st * P], f8, tag="s")
                    # selector DMA for whole st
                    mm_b0 = st_runs[0]["mm_base"]
                    nc.sync.dma_start(sel_sb[:, 0:st_nmm * P],
                                      sel_in_ap[:, mm_b0 * P:(mm_b0 + st_nmm) * P])
                    # gathers per run
                    blk_off = 0
                    for run in st_runs:
                        ns = run["n_slots"]
                        if ns == 0:
                            continue
                        wbase = run["w"] * WIN
                        win_rows = min(32768, N - wbase)
                        sb0 = run["slot_base"]
                        nb = ns // 128
                        nc.gpsimd.dma_gather(
                            out_ap=g_sb[:, blk_off * C:(blk_off + nb) * C].rearrange("p (b c) -> p b c", c=C),
                            in_ap=table[wbase:wbase + win_rows, :],
                            idxs_ap=idx_sb[:, sb0 // 16:(sb0 + ns) // 16],
                            num_idxs=ns, num_idxs_reg=ns, elem_size=C,
                        )
                        blk_off += nb
                    # psum tiles: 4 banks x [128,512] = 8 node-tiles
                    accs = [cps.tile([P, 512], f32, space="PSUM", tag="agg", name=f"agg{_i}")
                            for _i in range((st_ntiles + 1) // 2)]
                    # mm bookkeeping
                    mm_seq = []
                    blk_off = 0
                    for run in st_runs:
                        for (b, ti) in run["mm_list"]:
                            mm_seq.append((blk_off + b, ti))
                        blk_off += run["n_slots"] // 128
                    last = {}
                    for i, (b, ti) in enumerate(mm_seq):
                        last[ti] = i
                    # structural self-edges: psum[t] = diag_t.T @ hw_own_t (start=True)
                    for tl in range(st_ntiles):
                        t = st_t0 + tl
                        rows = min(P, NPC - t * P)
                        hwl = epi.tile([P, C], f16, tag="hwl")
                        if rows < P:
                            nc.vector.memset(hwl[:], 0.0)
                        nc.sync.dma_start(hwl[:rows, :], agin[t * P: t * P + rows, :])
                        dg = epi.tile([P, P], f8, tag="dg")
                        nc.sync.dma_start(dg[:], diag_in_ap[:, t * P:(t + 1) * P])
                        reg = accs[tl // 2][:, (tl % 2) * 256:(tl % 2) * 256 + 256]
                        # start=True clears the whole PSUM bank -> only the first
                        # matmul touching each bank (even tl) may set it
                        nc.tensor.matmul(reg, lhsT=dg[:], rhs=hwl[:],
                                         start=(tl % 2 == 0), stop=(last.get(t) is None),
                                         skip_group_check=True)
                    for i, (b, ti) in enumerate(mm_seq):
                        tl = ti - st_t0
                        reg = accs[tl // 2][:, (tl % 2) * 256:(tl % 2) * 256 + 256]
                        nc.tensor.matmul(
                            reg, lhsT=sel_sb[:, i * P:(i + 1) * P],
                            rhs=g_sb[:, b * C:(b + 1) * C],
                            start=False, stop=(last[ti] == i),
                            skip_group_check=True,
                        )
                    if DBG and layer == 1 and sti == 0:
                        nc.sync.dma_start(dbg_g1[:, 0:min(st_nblk, 64) * C], g_sb[:, 0:min(st_nblk, 64) * C])
                    # epilogue per node-tile
                    for tl in range(st_ntiles):
                        t = st_t0 + tl
                        reg = accs[tl // 2][:, (tl % 2) * 256:(tl % 2) * 256 + 256]
                        if DBG and layer == 1:
                            aggcp = epi.tile([P, C], f32, tag="aggcp")
                            nc.vector.tensor_copy(aggcp[:], reg)
                            nc.sync.dma_start(dbg_agg1[:, t * C:(t + 1) * C], aggcp[:])
                        tmp = epi.tile([P, C], f32, tag="etmp")
                        nc.vector.scalar_tensor_tensor(
                            out=tmp[:], in0=reg, scalar=invdeg_sb[:, t:t + 1],
                            in1=z_sb[:, t * C:(t + 1) * C], op0=OP.mult, op1=OP.add)
                        if DBG and layer == 1:
                            nc.sync.dma_start(dbg_pre1[:, t * C:(t + 1) * C], tmp[:])
                        nc.scalar.activation(z_sb[:, t * C:(t + 1) * C], tmp[:], AF.Relu,
                                             accum_out=s1_all[:, t:t + 1])
                        sq = epi.tile([P, C], f16, tag="esq")
                        nc.scalar.activation(sq[:], z_sb[:, t * C:(t + 1) * C], AF.Square,
                                             accum_out=s2_all[:, t:t + 1])
            # stats: mu, rstd over all tiles
            nc.vector.tensor_scalar(out=mu_all[:], in0=s1_all[:], scalar1=1.0 / C, scalar2=None, op0=OP.mult)
            var = small.tile([P, TILES], f32, tag="var")
            nc.vector.tensor_tensor(out=var[:], in0=mu_all[:], in1=mu_all[:], op=OP.mult)
            nc.vector.scalar_tensor_tensor(out=var[:], in0=s2_all[:], scalar=1.0 / C, in1=var[:],
                                           op0=OP.mult, op1=OP.subtract)
            std = small.tile([P, TILES], f32, tag="std")
            nc.vector.tensor_scalar(out=var[:], in0=var[:], scalar1=float(LN_EPS), scalar2=None, op0=OP.add)
            nc.scalar.activation(std[:], var[:], AF.Sqrt)
            nc.vector.reciprocal(rstd_all[:], std[:])
            # normalize -> h_sb fp16
            for t in range(TILES):
                nc.vector.tensor_scalar(
                    out=h_sb[:, t * C:(t + 1) * C], in0=z_sb[:, t * C:(t + 1) * C],
                    scalar1=mu_all[:, t:t + 1], scalar2=rstd_all[:, t:t + 1],
                    op0=OP.subtract, op1=OP.mult)

        # ---------------- layer 1
        dense_phase(1)
        if DBG:
            nc.sync.dma_start(dbg_z1[:], z_sb[:])
        nc.gpsimd.collective_compute(
            "AllGather", mybir.AluOpType.bypass,
            replica_groups=[list(range(NCORES))],
            ins=[ag_in[:].opt()], outs=[table1[:].opt()])
        gather_phase(1, lay1, table1, idx1_sb, sel1_in, invdeg1_sb, diag1_in, ag_in)
        if DBG:
            nc.sync.dma_start(dbg_h1[:], h_sb[:])
        # ---------------- layer 2
        dense_phase(2)
        nc.gpsimd.collective_compute(
            "AllGather", mybir.AluOpType.bypass,
            replica_groups=[list(range(NCORES))],
            ins=[ag_in2[:].opt()], outs=[table2[:].opt()])
        gather_phase(2, lay2, table2, idx2_sb, sel2_in, invdeg2_sb, diag2_in, ag_in2)
        # ---------------- final projection
        with tc.tile_pool(name="fps", bufs=4, space="PSUM") as fps, \
             tc.tile_pool(name="ftr", bufs=4, space="PSUM") as ftr, \
             tc.tile_pool(name="fe", bufs=4) as fe:
            for t in range(TILES):
                hT = fe.tile([P, 2, P], f16, tag="fhT")
                for k in range(2):
                    tp = ftr.tile([P, P], f16, space="PSUM", tag="ftp")
                    nc.tensor.transpose(tp[:], h_sb[:, t * C + k * P: t * C + (k + 1) * P], ident16[:])
                    nc.vector.tensor_copy(hT[:, k, :], tp[:])
                acc = fps.tile([P, OUT], f32, space="PSUM")
                nc.tensor.matmul(acc[:], lhsT=hT[:, 0, :], rhs=rhsF_sb[:, 0, :], start=True, stop=False)
                nc.tensor.matmul(acc[:], lhsT=hT[:, 1, :], rhs=rhsF_sb[:, 1, :], start=False, stop=False)
                nc.tensor.matmul(acc[:], lhsT=ones_col[:], rhs=browF_sb[:],
                                 start=False, stop=True, skip_group_check=True)
                o_sb = fe.tile([P, OUT], f32, tag="fo")
                nc.vector.tensor_copy(o_sb[:], acc[:])
                nc.sync.dma_start(out_dram[t * P:(t + 1) * P, :], o_sb[:])

    nc.compile()
    return nc


# ---------------------------------------------------------------- entry point
def kernel(**inputs):
    from concourse.bass_utils import run_bass_kernel_spmd

    in_maps, lay1, lay2 = _prep(inputs)
    key = "nc"
    if key not in _COMPILED:
        _COMPILED[key] = _build_nc(lay1, lay2)
    nc = _COMPILED[key]
    res = run_bass_kernel_spmd(nc, in_maps, core_ids=list(range(NCORES)))
    _COMPILED["last_res"] = res
    out = np.concatenate([res.results[c]["out"][:NPC] for c in range(NCORES)], axis=0)
    return out.astype(np.float32)



# revision 28
# speedup vs baseline: 56.7308x; 56.7308x over previous
"""MetaPathGNN forward on 8 Trainium2 NeuronCores (Bass/Tile).

Strategy (self-contained; shapes hardcoded for N=100000, C=256, OUT=128, E=400000):
  - Nodes sharded 12500/core (padded 12544). Per layer: each core computes
    hw = h @ wlT (fp16) for its nodes; the per-core hw rows are split in two
    halves (48 tiles / 50 tiles) and exchanged with TWO AllGathers so the
    first collective overlaps the second half of the dense phase and the
    second collective overlaps the first gather pass.
  - Message table layout: [secA | secB], secA = 8 cores x 6144 rows,
    secB = 8 cores x 6400 rows. Four int16 gather windows (2 per section).
  - Edges assigned to cores by src owner; host sorts each core's edges by
    (pass, super-tile(src), window(dst), tile(src), src) and pads so the slot
    layout is identical across cores (single SPMD NEFF).
  - Messages gathered with GpSimd dma_gather in PREPARE_ONLY mode +
    trigger_dma on 4 SWDGE queues: descriptor prep pipelines with the DMA
    drains instead of serializing on GpSimd.
  - Segment-sum = fp8(0/1 selector) x fp16(messages) matmuls accumulated in
    PSUM. Pass A (windows 0,1) drains into z via (psum*invdeg + z); pass B
    (windows 2,3) starts from structural self-edge (diag) matmuls and its
    epilogue fuses deg-normalize + dense term, relu + LN stats, with LN
    normalization done per super-tile so it pipelines.
  - Dense terms computed as fp16 matmuls; lhs loads batched 4 tiles/DMA,
    hw bounce stores batched 8 tiles/DMA.
"""
import os
import numpy as np
from contextlib import ExitStack

N = 100000
C = 256
OUT = 128
NCORES = 8
NPC = N // NCORES          # 12500 nodes per core
P = 128
TILES = (NPC + P - 1) // P  # 98
NPC_PAD = TILES * P         # 12544
ST_TILES = 8                # node-tiles per super-tile
NST = (TILES + ST_TILES - 1) // ST_TILES  # 13
# 4 st-aligned per-core sections -> 4 chunked AllGathers, window == section
SEC_ST = ((0, 3), (3, 6), (6, 9), (9, 13))   # st ranges per section
SROWS = (3072, 3072, 3072, 3328)             # rows per core per section
SB = (0, 3072, 6144, 9216)                   # per-core row offset of section
WSIZE = tuple(NCORES * r for r in SROWS)     # (24576, 24576, 24576, 26624)
WBASE = (0, 24576, 49152, 73728)             # global table row offsets
NWIN = 4
LN_EPS = 1e-5

_COMPILED = {}


# ---------------------------------------------------------------- host side
def _sigmoid(x):
    return 1.0 / (1.0 + np.exp(-np.float64(x)))


def _pos_of(r):
    """Global node id -> message-table row under the four-section layout."""
    c = r // NPC
    o = r % NPC
    k = np.digitize(o, SB[1:])
    kb = np.asarray(WBASE)[k]
    return kb + c * np.asarray(SROWS)[k] + (o - np.asarray(SB)[k])


def _build_layer(src, dst):
    """Vectorized layout builder. Slot stream is ordered (pass, st, w, tile, s)
    so pass A (windows 0,1) is a contiguous prefix. Returns dict with:
       runs: flat list of run dicts (pass, sti, w, n_slots, mm_list,
             slot_base, mm_base) in stream order
       idx:  [NCORES, 128, S/16] int16 window-local gather indices
       sel:  [NCORES, 128, NMM*128] selector blobs
       diag: [NCORES, 128, TILES*128] self-edge multiplicity blobs
    """
    per_core = []
    diag = np.zeros((NCORES, 128, TILES * 128), dtype=np.uint8)
    for c in range(NCORES):
        lo = c * NPC
        m = (src >= lo) & (src < lo + NPC)
        # structural self-edges bypass the gather: counted into a diagonal blob
        selfm = m & (src == dst)
        si = (src[selfm] - lo).astype(np.int64)
        mult = np.bincount(si, minlength=NPC_PAD)
        pp = np.arange(NPC_PAD)
        diag[c, pp & 127, (pp >> 7) * 128 + (pp & 127)] = mult
        m = m & (src != dst)
        s = (src[m] - lo).astype(np.int64)
        d = dst[m].astype(np.int64)
        pos = _pos_of(d)
        w = np.digitize(pos, WBASE[1:])  # window index 0..3
        t = s >> 7
        sti = t >> 3
        wp = w >> 1                      # pass (0: windows 0/1, 1: windows 2/3)
        order = np.lexsort((s, t, w, sti, wp))
        per_core.append((s[order], (pos - np.asarray(WBASE)[w])[order],
                         w[order], t[order], sti[order]))

    cnt = np.zeros((NCORES, NST, NWIN, TILES), dtype=np.int64)
    for c in range(NCORES):
        s, d, w, t, sti = per_core[c]
        np.add.at(cnt[c], (sti, w, t), 1)
    ucnt = cnt.max(axis=0)

    runs = []
    total_slots = 0
    total_mms = 0
    slot_tile_all = []
    seg_start = {}  # (st,w,tile) -> unified slot start position
    for wp in range(2):
        for sti in range(NST):
            for wi in (2 * wp, 2 * wp + 1):
                segs = [(ti, int(ucnt[sti, wi, ti]))
                        for ti in range(sti * ST_TILES, min((sti + 1) * ST_TILES, TILES))
                        if ucnt[sti, wi, ti] > 0]
                maxtpb = int(os.environ.get("K_MAXTPB", "4"))
                slots = []
                tiles_in_blk = set()
                f = 0
                for (ti, n) in segs:
                    if f > 0 and len(tiles_in_blk) >= maxtpb and ti not in tiles_in_blk:
                        slots.extend([-1] * (128 - f))
                        f = 0
                        tiles_in_blk = set()
                    seg_start[(sti, wi, ti)] = total_slots + len(slots)
                    rem = n
                    while rem > 0:
                        take = min(128 - f, rem)
                        slots.extend([ti] * take)
                        f += take
                        rem -= take
                        tiles_in_blk.add(ti)
                        if f == 128:
                            f = 0
                            tiles_in_blk = set()
                if f > 0:
                    slots.extend([-1] * (128 - f))
                n_slots = len(slots)
                nblk = n_slots // 128
                mm_list = []
                for b in range(nblk):
                    blk = slots[b * 128:(b + 1) * 128]
                    touched = []
                    for q in blk:
                        if q >= 0 and q not in touched:
                            touched.append(q)
                    for ti in touched:
                        mm_list.append((b, ti))
                runs.append({"pass": wp, "sti": sti, "w": wi, "n_slots": n_slots,
                             "mm_list": mm_list, "slot_base": total_slots,
                             "mm_base": total_mms})
                slot_tile_all.extend(slots)
                total_slots += n_slots
                total_mms += len(mm_list)

    slot_tile_all = np.asarray(slot_tile_all, dtype=np.int64)

    # per-core: place edges into unified slots
    idx = np.zeros((NCORES, total_slots), dtype=np.int16)
    selcol = np.full((NCORES, total_slots), -1, dtype=np.int64)  # src col (s%128)
    for c in range(NCORES):
        s, d, w, t, sti = per_core[c]
        wp = w >> 1
        key = (((wp * NST + sti) * NWIN) + w) * TILES + t
        change = np.empty(len(key), dtype=bool)
        if len(key):
            change[0] = True
            change[1:] = key[1:] != key[:-1]
        grp_start_idx = np.flatnonzero(change)
        grp_of_edge = np.cumsum(change) - 1
        offset_in_grp = np.arange(len(key)) - grp_start_idx[grp_of_edge]
        base = np.array([seg_start[(int(sti[i]), int(w[i]), int(t[i]))]
                         for i in grp_start_idx], dtype=np.int64)
        slot_pos = base[grp_of_edge] + offset_in_grp
        idx[c, slot_pos] = d.astype(np.int16)
        selcol[c, slot_pos] = s & 127

    # selector blob: [128 partitions(slot within block), total_mms*128]
    sel = np.zeros((NCORES, 128, total_mms * 128), dtype=np.uint8)
    mm_i_global = 0
    for run in runs:
        sb = run["slot_base"]
        for (b, ti) in run["mm_list"]:
            sl0 = sb + b * 128
            tile_match = slot_tile_all[sl0:sl0 + 128] == ti
            for c in range(NCORES):
                cols = selcol[c, sl0:sl0 + 128]
                jj = np.flatnonzero(tile_match & (cols >= 0))
                sel[c, jj, mm_i_global * 128 + cols[jj]] = 1
            mm_i_global += 1
    assert mm_i_global == total_mms

    # wrapped idx arrays: [128, total_slots/16]; slot i -> [i%16 (+16g), i//16]
    assert total_slots % 128 == 0
    idx_w = np.zeros((NCORES, 16, total_slots // 16), dtype=np.int16)
    ar = np.arange(total_slots)
    idx_w[:, ar % 16, ar // 16] = idx
    idx_w = np.tile(idx_w, (1, 8, 1))

    return {"runs": runs, "total_slots": total_slots, "total_mms": total_mms,
            "idx": idx_w, "sel": sel, "diag": diag}


def _prep(inputs):
    """All host-side preprocessing -> per-core input maps + static meta."""
    import ml_dtypes
    f16 = np.float16
    x = np.asarray(inputs["x"], np.float32)
    ei1 = np.asarray(inputs["edge_index_r1"])
    ei0 = np.asarray(inputs["edge_index_r0"])

    g1 = np.float32(_sigmoid(inputs["gate1"]))
    g0 = np.float32(_sigmoid(inputs["gate0"]))
    lns1 = np.asarray(inputs["lns1"], np.float32); lnb1 = np.asarray(inputs["lnb1"], np.float32)
    lns0 = np.asarray(inputs["lns0"], np.float32); lnb0 = np.asarray(inputs["lnb0"], np.float32)

    wl1 = np.asarray(inputs["wl1"], np.float32); bl1 = np.asarray(inputs["bl1"], np.float32)
    w01 = np.asarray(inputs["w01"], np.float32); b01 = np.asarray(inputs["b01"], np.float32)
    w11 = np.asarray(inputs["w11"], np.float32); b11 = np.asarray(inputs["b11"], np.float32)
    wl0 = np.asarray(inputs["wl0"], np.float32); bl0 = np.asarray(inputs["bl0"], np.float32)
    w00 = np.asarray(inputs["w00"], np.float32); b00 = np.asarray(inputs["b00"], np.float32)
    w10 = np.asarray(inputs["w10"], np.float32); b10 = np.asarray(inputs["b10"], np.float32)
    Wout = np.asarray(inputs["Wout"], np.float32); bout = np.asarray(inputs["bout"], np.float32)

    # Layer 1 (uses r1 edges, params *1); h = x0 = x
    rhsAB1 = np.concatenate([wl1.T, ((1 - g1) * w01 + g1 * w11).T], axis=1)  # [256,512]
    bias1 = bl1 + (1 - g1) * b01 + g1 * b11                                   # [256]
    # Layer 2 (r0 edges, params *0); h = h1 = h1r*lns1 + lnb1, x0 = x
    wl0T_s = (lns1[:, None] * wl0.T)            # [256,256] for h1r path
    const1 = lnb1 @ wl0.T                        # [256] message const
    w00T_s = (1 - g0) * (lns1[:, None] * w00.T)
    rhsA2 = np.concatenate([wl0T_s, w00T_s], axis=1)  # [256,512]
    w10T_s = g0 * w10.T                          # [256,256] x path
    bias2 = bl0 + (1 - g0) * (b00 + lnb1 @ w00.T) + g0 * b10
    crow2 = np.concatenate([const1, bias2])      # [512]
    # Final: out = h2 @ WoutT + bout, h2 = h2r*lns0 + lnb0
    WoutT_s = lns0[:, None] * Wout.T             # [256,128]
    bout_s = bout + lnb0 @ Wout.T                # [128]

    # degree normalization (bincount over dst of the FULL edge list)
    inv1 = 1.0 / np.clip(np.bincount(ei1[1], minlength=N), 1.0, None).astype(np.float32)
    inv0 = 1.0 / np.clip(np.bincount(ei0[1], minlength=N), 1.0, None).astype(np.float32)

    lay1 = _build_layer(ei1[0].astype(np.int64), ei1[1].astype(np.int64))
    lay2 = _build_layer(ei0[0].astype(np.int64), ei0[1].astype(np.int64))

    fp8 = ml_dtypes.float8_e4m3
    in_maps = []
    for c in range(NCORES):
        lo = c * NPC
        xs = np.zeros((NPC_PAD, C), np.float32)
        xs[:NPC] = x[lo:lo + NPC]
        inv1c = np.zeros(NPC_PAD, np.float32); inv1c[:NPC] = inv1[lo:lo + NPC]
        inv0c = np.zeros(NPC_PAD, np.float32); inv0c[:NPC] = inv0[lo:lo + NPC]
        in_maps.append(dict(
            xT=np.ascontiguousarray(xs.T).astype(f16),
            rhsAB1=rhsAB1.astype(f16), brow1=bias1[None, :].astype(f16),
            rhsA2=rhsA2.astype(f16), rhsY2=w10T_s.astype(f16), crow2=crow2[None, :].astype(f16),
            rhsF=WoutT_s.astype(f16), browF=bout_s[None, :].astype(f16),
            invdeg1=np.ascontiguousarray(inv1c.reshape(TILES, P).T),
            invdeg2=np.ascontiguousarray(inv0c.reshape(TILES, P).T),
            idx1=lay1["idx"][c], idx2=lay2["idx"][c],
            sel1=lay1["sel"][c].astype(np.float32).astype(fp8),
            sel2=lay2["sel"][c].astype(np.float32).astype(fp8),
            diag1=lay1["diag"][c].astype(np.float32).astype(fp8),
            diag2=lay2["diag"][c].astype(np.float32).astype(fp8),
        ))
    return in_maps, lay1, lay2


# ---------------------------------------------------------------- device side
def _build_nc(lay1, lay2):
    import concourse.bass as bass
    import concourse.tile as tile
    from concourse import bacc, mybir
    from concourse.masks import make_identity

    f32, f16 = mybir.dt.float32, mybir.dt.float16
    f8, i16 = mybir.dt.float8e4, mybir.dt.int16
    AF = mybir.ActivationFunctionType
    OP = mybir.AluOpType

    nc = bacc.Bacc("TRN2", target_bir_lowering=False, debug=False,
                   num_devices=NCORES, num_swdge_queues=4,
                   dynamic_dma_scratch_size=49152)

    S1, S2 = lay1["total_slots"], lay2["total_slots"]
    M1, M2 = lay1["total_mms"], lay2["total_mms"]

    xT_in = nc.dram_tensor("xT", [C, NPC_PAD], f16, kind="ExternalInput").ap()
    rhsAB1_in = nc.dram_tensor("rhsAB1", [C, 512], f16, kind="ExternalInput").ap()
    brow1_in = nc.dram_tensor("brow1", [1, 256], f16, kind="ExternalInput").ap()
    rhsA2_in = nc.dram_tensor("rhsA2", [C, 512], f16, kind="ExternalInput").ap()
    rhsY2_in = nc.dram_tensor("rhsY2", [C, 256], f16, kind="ExternalInput").ap()
    crow2_in = nc.dram_tensor("crow2", [1, 512], f16, kind="ExternalInput").ap()
    rhsF_in = nc.dram_tensor("rhsF", [C, OUT], f16, kind="ExternalInput").ap()
    browF_in = nc.dram_tensor("browF", [1, OUT], f16, kind="ExternalInput").ap()
    invdeg1_in = nc.dram_tensor("invdeg1", [P, TILES], f32, kind="ExternalInput").ap()
    invdeg2_in = nc.dram_tensor("invdeg2", [P, TILES], f32, kind="ExternalInput").ap()
    idx1_in = nc.dram_tensor("idx1", [P, S1 // 16], i16, kind="ExternalInput").ap()
    idx2_in = nc.dram_tensor("idx2", [P, S2 // 16], i16, kind="ExternalInput").ap()
    sel1_in = nc.dram_tensor("sel1", [P, M1 * P], f8, kind="ExternalInput").ap()
    sel2_in = nc.dram_tensor("sel2", [P, M2 * P], f8, kind="ExternalInput").ap()
    diag1_in = nc.dram_tensor("diag1", [P, TILES * P], f8, kind="ExternalInput").ap()
    diag2_in = nc.dram_tensor("diag2", [P, TILES * P], f8, kind="ExternalInput").ap()

    out_dram = nc.dram_tensor("out", [NPC_PAD, OUT], f32, kind="ExternalOutput").ap()

    # separate DRAM tensors per table section so each window's gathers depend
    # only on its own chunked AllGather (DRAM dep tracking is per-tensor)
    ags = {(l, k): nc.dram_tensor(f"ag{l}_{k}", [SROWS[k], C], f16)
           for l in (1, 2) for k in range(4)}
    tabs = {(l, k): nc.dram_tensor(f"tab{l}_{k}", [WSIZE[k], C], f16,
                                   addr_space="Shared")
            for l in (1, 2) for k in range(4)}

    # per-(st, pass) sizing for gather/selector staging tiles
    def _st_sizes(lay):
        blk = np.zeros((2, NST), dtype=np.int64)
        mmn = np.zeros((2, NST), dtype=np.int64)
        for run in lay["runs"]:
            blk[run["pass"], run["sti"]] += run["n_slots"] // 128
            mmn[run["pass"], run["sti"]] += len(run["mm_list"])
        return blk, mmn

    blk1, mmn1 = _st_sizes(lay1)
    blk2, mmn2 = _st_sizes(lay2)
    max_blk = int(max(blk1.max(), blk2.max()))
    max_mm = int(max(mmn1.max(), mmn2.max()))

    with tile.TileContext(nc) as tc, ExitStack() as ctx:
        sb = ctx.enter_context(tc.tile_pool(name="sb", bufs=1))
        lhs_pool = ctx.enter_context(tc.tile_pool(name="lhs", bufs=2))
        hwst_pool = ctx.enter_context(tc.tile_pool(name="hwst", bufs=2))
        small = ctx.enter_context(tc.tile_pool(name="small", bufs=2))
        gpool = ctx.enter_context(tc.tile_pool(name="gst", bufs=2))
        spool = ctx.enter_context(tc.tile_pool(name="sel", bufs=2))
        dpool = ctx.enter_context(tc.tile_pool(name="dg", bufs=2))
        epi = ctx.enter_context(tc.tile_pool(name="epi", bufs=4))

        # persistent staging
        z_sb = sb.tile([P, TILES * C], f16)      # dense term, then relu output r
        h_sb = sb.tile([P, TILES * C], f16)      # LN output (h1r, then h2r)
        s1_all = sb.tile([P, TILES], f32)
        s2_all = sb.tile([P, TILES], f32)
        invdeg1_sb = sb.tile([P, TILES], f32)
        invdeg2_sb = sb.tile([P, TILES], f32)
        ident16 = sb.tile([P, P], f16)
        make_identity(nc, ident16[:])
        ones_col = sb.tile([1, P], f16)
        nc.vector.memset(ones_col[:], 1.0)
        nc.sync.dma_start(invdeg1_sb[:], invdeg1_in[:])
        nc.sync.dma_start(invdeg2_sb[:], invdeg2_in[:])

        # weight tiles (persistent)
        rhsAB1_sb = sb.tile([C // 2, 2, 512], f16)
        nc.sync.dma_start(rhsAB1_sb[:], rhsAB1_in[:].rearrange("(b k) n -> k b n", k=128))
        brow1_sb = sb.tile([1, 256], f16)
        nc.sync.dma_start(brow1_sb[:], brow1_in[:])
        rhsA2_sb = sb.tile([C // 2, 2, 512], f16)
        nc.sync.dma_start(rhsA2_sb[:], rhsA2_in[:].rearrange("(b k) n -> k b n", k=128))
        rhsY2_sb = sb.tile([C // 2, 2, 256], f16)
        nc.sync.dma_start(rhsY2_sb[:], rhsY2_in[:].rearrange("(b k) n -> k b n", k=128))
        crow2_sb = sb.tile([1, 512], f16)
        nc.sync.dma_start(crow2_sb[:], crow2_in[:])
        rhsF_sb = sb.tile([C // 2, 2, OUT], f16)
        nc.sync.dma_start(rhsF_sb[:], rhsF_in[:].rearrange("(b k) n -> k b n", k=128))
        browF_sb = sb.tile([1, OUT], f16)
        nc.sync.dma_start(browF_sb[:], browF_in[:])

        idx1_sb = sb.tile([P, S1 // 16], i16)
        nc.sync.dma_start(idx1_sb[:], idx1_in[:])
        idx2_sb = sb.tile([P, S2 // 16], i16)
        nc.sync.dma_start(idx2_sb[:], idx2_in[:])

        def dense_phase(layer):
            """Compute [hw | z] per tile; hw -> sectioned ag bounces, z -> z_sb."""
            with tc.tile_pool(name="abps", bufs=4, space="PSUM") as abps, \
                 tc.tile_pool(name="trps", bufs=4, space="PSUM") as trps, \
                 tc.tile_pool(name="abe", bufs=4) as abe:
                lh4 = None
                stage = None
                for t in range(TILES):
                    if t % 4 == 0:
                        nt = min(4, TILES - t)
                        lh4 = lhs_pool.tile([P, 2, 4 * P], f16, tag="xT4")
                        nc.sync.dma_start(
                            lh4[:, :, 0:nt * P],
                            xT_in[:, t * P:(t + nt) * P].rearrange("(b k) n -> k b n", k=128))
                    j = (t % 4) * P
                    acc = abps.tile([P, 512], f32, space="PSUM")
                    if layer == 1:
                        nc.tensor.matmul(acc[:], lhsT=lh4[:, 0, j:j + P], rhs=rhsAB1_sb[:, 0, :], start=True, stop=False)
                        nc.tensor.matmul(acc[:], lhsT=lh4[:, 1, j:j + P], rhs=rhsAB1_sb[:, 1, :], start=False, stop=False)
                        nc.tensor.matmul(acc[:, 256:512], lhsT=ones_col[:], rhs=brow1_sb[:],
                                         start=False, stop=True, skip_group_check=True)
                    else:
                        # transpose h1r tile -> lhsT fp16
                        hT = abe.tile([P, 2, P], f16, tag="hT")
                        for k in range(2):
                            tp = trps.tile([P, P], f16, space="PSUM", tag="trp")
                            nc.tensor.transpose(tp[:], h_sb[:, t * C + k * P: t * C + (k + 1) * P], ident16[:])
                            nc.vector.tensor_copy(hT[:, k, :], tp[:])
                        nc.tensor.matmul(acc[:], lhsT=hT[:, 0, :], rhs=rhsA2_sb[:, 0, :], start=True, stop=False)
                        nc.tensor.matmul(acc[:], lhsT=hT[:, 1, :], rhs=rhsA2_sb[:, 1, :], start=False, stop=False)
                        nc.tensor.matmul(acc[:, 256:512], lhsT=lh4[:, 0, j:j + P], rhs=rhsY2_sb[:, 0, :],
                                         start=False, stop=False, skip_group_check=True)
                        nc.tensor.matmul(acc[:, 256:512], lhsT=lh4[:, 1, j:j + P], rhs=rhsY2_sb[:, 1, :],
                                         start=False, stop=False, skip_group_check=True)
                        nc.tensor.matmul(acc[:], lhsT=ones_col[:], rhs=crow2_sb[:],
                                         start=False, stop=True, skip_group_check=True)
                    # hw half -> fp16 staging; one bounce DMA per super-tile
                    if t % ST_TILES == 0:
                        stage = hwst_pool.tile([P, ST_TILES * C], f16, tag="hwstage")
                    nc.scalar.activation(stage[:, (t % ST_TILES) * C:(t % ST_TILES + 1) * C],
                                         acc[:, 0:256], AF.Copy)
                    if t % ST_TILES == ST_TILES - 1 or t == TILES - 1:
                        st = t // ST_TILES
                        t0 = st * ST_TILES
                        ntl = t - t0 + 1
                        k = min(st // 3, 3)
                        dst = ags[(layer, k)][t0 * P - SB[k]: t0 * P - SB[k] + ntl * P, :]
                        nc.sync.dma_start(
                            dst.rearrange("(b k) n -> k b n", k=128),
                            stage[:, 0:ntl * C].rearrange("p (b c) -> p b c", c=C))
                        if st == SEC_ST[k][1] - 1:
                            # section complete -> kick its AllGather now
                            nc.gpsimd.collective_compute(
                                "AllGather", mybir.AluOpType.bypass,
                                replica_groups=[list(range(NCORES))],
                                ins=[ags[(layer, k)][:].opt()],
                                outs=[tabs[(layer, k)][:].opt()])
                    # z half -> z_sb fp16
                    nc.vector.tensor_copy(z_sb[:, t * C:(t + 1) * C], acc[:, 256:512])

        def gather_pass(lnum, lay, idx_sb, sel_in_ap, invdeg_sb, diag_in_ap,
                        wp, cps):
            """One gather pass (wp=0: windows 0,1 drain into z; wp=1: windows
            2,3 + diag, epilogue with relu/LN)."""
            runs_by_st = {}
            for run in lay["runs"]:
                if run["pass"] == wp:
                    runs_by_st.setdefault(run["sti"], []).append(run)
            for sti in range(NST):
                st_runs = runs_by_st.get(sti, [])
                st_nblk = sum(r["n_slots"] for r in st_runs) // 128
                st_nmm = sum(len(r["mm_list"]) for r in st_runs)
                st_t0 = sti * ST_TILES
                st_ntiles = min(ST_TILES, TILES - st_t0)
                if st_nblk == 0 and wp == 0:
                    continue
                g_sb = None
                sel_sb = None
                if st_nblk > 0:
                    g_sb = gpool.tile([P, max_blk * C], f16, tag="g")
                    sel_sb = spool.tile([P, max_mm * P], f8, tag="s")
                    mm_b0 = st_runs[0]["mm_base"]
                    nc.sync.dma_start(sel_sb[:, 0:st_nmm * P],
                                      sel_in_ap[:, mm_b0 * P:(mm_b0 + st_nmm) * P])
                    # gathers per run: PREPARE_ONLY + trigger, round-robin queues
                    blk_off = 0
                    for ri, run in enumerate(st_runs):
                        ns = run["n_slots"]
                        if ns == 0:
                            continue
                        wi = run["w"]
                        sb0 = run["slot_base"]
                        nb = ns // 128
                        gout = g_sb[:, blk_off * C:(blk_off + nb) * C].rearrange("p (b c) -> p b c", c=C)
                        gidx = idx_sb[:, sb0 // 16:(sb0 + ns) // 16]
                        nc.gpsimd.dma_gather(
                            out_ap=gout, in_ap=tabs[(lnum, wi)][:],
                            idxs_ap=gidx, num_idxs=ns, num_idxs_reg=ns,
                            elem_size=C,
                        )
                        blk_off += nb
                # psum tiles: 4 banks x [128,512] = 8 node-tiles
                accs = [cps.tile([P, 512], f32, space="PSUM", tag="agg", name=f"agg{wp}_{sti}_{_i}")
                        for _i in range((st_ntiles + 1) // 2)]
                # mm bookkeeping
                mm_seq = []
                blk_off = 0
                for run in st_runs:
                    for (b, ti) in run["mm_list"]:
                        mm_seq.append((blk_off + b, ti))
                    blk_off += run["n_slots"] // 128
                last = {}
                first_of_pair = {}
                for i, (b, ti) in enumerate(mm_seq):
                    last[ti] = i
                    pr = (ti - st_t0) // 2
                    if pr not in first_of_pair:
                        first_of_pair[pr] = i
                if wp == 1:
                    # structural self-edges clear every bank: diag first
                    dg = dpool.tile([P, ST_TILES * P], f8, tag="dg")
                    nc.sync.dma_start(dg[:, 0:st_ntiles * P],
                                      diag_in_ap[:, st_t0 * P:(st_t0 + st_ntiles) * P])
                    hwl = dpool.tile([P, ST_TILES * C], f16, tag="hwl")
                    k = min(sti // 3, 3)
                    src = ags[(lnum, k)][st_t0 * P - SB[k]: st_t0 * P - SB[k] + st_ntiles * P, :]
                    nc.sync.dma_start(hwl[:, 0:st_ntiles * C].rearrange("p (b c) -> p b c", c=C),
                                      src.rearrange("(b k) n -> k b n", k=128))
                    for tl in range(st_ntiles):
                        t = st_t0 + tl
                        reg = accs[tl // 2][:, (tl % 2) * 256:(tl % 2) * 256 + 256]
                        nc.tensor.matmul(reg, lhsT=dg[:, tl * P:(tl + 1) * P],
                                         rhs=hwl[:, tl * C:(tl + 1) * C],
                                         start=(tl % 2 == 0), stop=(last.get(t) is None),
                                         skip_group_check=True)
                for i, (b, ti) in enumerate(mm_seq):
                    tl = ti - st_t0
                    reg = accs[tl // 2][:, (tl % 2) * 256:(tl % 2) * 256 + 256]
                    nc.tensor.matmul(
                        reg, lhsT=sel_sb[:, i * P:(i + 1) * P],
                        rhs=g_sb[:, b * C:(b + 1) * C],
                        start=(wp == 0 and first_of_pair[(tl) // 2] == i),
                        stop=(last[ti] == i),
                        skip_group_check=True,
                    )
                if wp == 0:
                    # drain pass A: z += psum * invdeg  (only tiles with mms)
                    for tl in range(st_ntiles):
                        t = st_t0 + tl
                        if t not in last:
                            continue
                        reg = accs[tl // 2][:, (tl % 2) * 256:(tl % 2) * 256 + 256]
                        nc.vector.scalar_tensor_tensor(
                            out=z_sb[:, t * C:(t + 1) * C], in0=reg,
                            scalar=invdeg_sb[:, t:t + 1],
                            in1=z_sb[:, t * C:(t + 1) * C], op0=OP.mult, op1=OP.add)
                else:
                    # epilogue per node-tile + per-st LN
                    for tl in range(st_ntiles):
                        t = st_t0 + tl
                        reg = accs[tl // 2][:, (tl % 2) * 256:(tl % 2) * 256 + 256]
                        tmp = epi.tile([P, C], f32, tag="etmp")
                        nc.vector.scalar_tensor_tensor(
                            out=tmp[:], in0=reg, scalar=invdeg_sb[:, t:t + 1],
                            in1=z_sb[:, t * C:(t + 1) * C], op0=OP.mult, op1=OP.add)
                        nc.scalar.activation(z_sb[:, t * C:(t + 1) * C], tmp[:], AF.Relu,
                                             accum_out=s1_all[:, t:t + 1])
                        sq = epi.tile([P, C], f16, tag="esq")
                        nc.scalar.activation(sq[:], z_sb[:, t * C:(t + 1) * C], AF.Square,
                                             accum_out=s2_all[:, t:t + 1])
                    # per-st LN stats
                    sl = slice(st_t0, st_t0 + st_ntiles)
                    mu = small.tile([P, ST_TILES], f32, tag="mu")
                    var = small.tile([P, ST_TILES], f32, tag="var")
                    std = small.tile([P, ST_TILES], f32, tag="std")
                    rstd = small.tile([P, ST_TILES], f32, tag="rstd")
                    nn = st_ntiles
                    nc.vector.tensor_scalar(out=mu[:, 0:nn], in0=s1_all[:, sl], scalar1=1.0 / C,
                                            scalar2=None, op0=OP.mult)
                    nc.vector.tensor_tensor(out=var[:, 0:nn], in0=mu[:, 0:nn], in1=mu[:, 0:nn], op=OP.mult)
                    nc.vector.scalar_tensor_tensor(out=var[:, 0:nn], in0=s2_all[:, sl], scalar=1.0 / C,
                                                   in1=var[:, 0:nn], op0=OP.mult, op1=OP.subtract)
                    nc.vector.tensor_scalar(out=var[:, 0:nn], in0=var[:, 0:nn], scalar1=float(LN_EPS),
                                            scalar2=None, op0=OP.add)
                    nc.scalar.activation(std[:, 0:nn], var[:, 0:nn], AF.Sqrt)
                    nc.vector.reciprocal(rstd[:, 0:nn], std[:, 0:nn])
                    for tl in range(st_ntiles):
                        t = st_t0 + tl
                        nc.vector.tensor_scalar(
                            out=h_sb[:, t * C:(t + 1) * C], in0=z_sb[:, t * C:(t + 1) * C],
                            scalar1=mu[:, tl:tl + 1], scalar2=rstd[:, tl:tl + 1],
                            op0=OP.subtract, op1=OP.mult)

        def layer(lnum, lay, idx_sb, sel_in, invdeg_sb, diag_in):
            dense_phase(lnum)  # kicks the 4 chunked AllGathers as sections finish
            with tc.tile_pool(name=f"cpsA{lnum}", bufs=8, space="PSUM") as cpsA:
                gather_pass(lnum, lay, idx_sb, sel_in, invdeg_sb, diag_in, 0, cpsA)
            with tc.tile_pool(name=f"cpsB{lnum}", bufs=8, space="PSUM") as cpsB:
                gather_pass(lnum, lay, idx_sb, sel_in, invdeg_sb, diag_in, 1, cpsB)

        # ---------------- layer 1
        layer(1, lay1, idx1_sb, sel1_in, invdeg1_sb, diag1_in)
        # ---------------- layer 2
        layer(2, lay2, idx2_sb, sel2_in, invdeg2_sb, diag2_in)
        # ---------------- final projection
        with tc.tile_pool(name="fps", bufs=4, space="PSUM") as fps, \
             tc.tile_pool(name="ftr", bufs=4, space="PSUM") as ftr, \
             tc.tile_pool(name="fe", bufs=4) as fe:
            for t in range(TILES):
                hT = fe.tile([P, 2, P], f16, tag="fhT")
                for k in range(2):
                    tp = ftr.tile([P, P], f16, space="PSUM", tag="ftp")
                    nc.tensor.transpose(tp[:], h_sb[:, t * C + k * P: t * C + (k + 1) * P], ident16[:])
                    nc.vector.tensor_copy(hT[:, k, :], tp[:])
                acc = fps.tile([P, OUT], f32, space="PSUM")
                nc.tensor.matmul(acc[:], lhsT=hT[:, 0, :], rhs=rhsF_sb[:, 0, :], start=True, stop=False)
                nc.tensor.matmul(acc[:], lhsT=hT[:, 1, :], rhs=rhsF_sb[:, 1, :], start=False, stop=False)
                nc.tensor.matmul(acc[:], lhsT=ones_col[:], rhs=browF_sb[:],
                                 start=False, stop=True, skip_group_check=True)
                o_sb = fe.tile([P, OUT], f32, tag="fo")
                nc.vector.tensor_copy(o_sb[:], acc[:])
                nc.sync.dma_start(out_dram[t * P:(t + 1) * P, :], o_sb[:])

    nc.compile()
    return nc


# ---------------------------------------------------------------- entry point
def kernel(**inputs):
    from concourse.bass_utils import run_bass_kernel_spmd

    in_maps, lay1, lay2 = _prep(inputs)
    key = "nc"
    if key not in _COMPILED:
        _COMPILED[key] = _build_nc(lay1, lay2)
    nc = _COMPILED[key]
    res = run_bass_kernel_spmd(nc, in_maps, core_ids=list(range(NCORES)))
    _COMPILED["last_res"] = res
    out = np.concatenate([res.results[c]["out"][:NPC] for c in range(NCORES)], axis=0)
    return out.astype(np.float32)


# revision 34
# speedup vs baseline: 62.8296x; 1.1075x over previous
"""MetaPathGNN forward on 8 Trainium2 NeuronCores (Bass/Tile).

Strategy (self-contained; shapes hardcoded for N=100000, C=256, OUT=128, E=400000):
  - Nodes sharded 12500/core (padded 12544). Per layer: each core computes
    hw = h @ wlT (fp16) for its nodes; the per-core hw rows are split in two
    halves (48 tiles / 50 tiles) and exchanged with TWO AllGathers so the
    first collective overlaps the second half of the dense phase and the
    second collective overlaps the first gather pass.
  - Message table layout: [secA | secB], secA = 8 cores x 6144 rows,
    secB = 8 cores x 6400 rows. Four int16 gather windows (2 per section).
  - Edges assigned to cores by src owner; host sorts each core's edges by
    (pass, super-tile(src), window(dst), tile(src), src) and pads so the slot
    layout is identical across cores (single SPMD NEFF).
  - Messages gathered with GpSimd dma_gather in PREPARE_ONLY mode +
    trigger_dma on 4 SWDGE queues: descriptor prep pipelines with the DMA
    drains instead of serializing on GpSimd.
  - Segment-sum = fp8(0/1 selector) x fp16(messages) matmuls accumulated in
    PSUM. Pass A (windows 0,1) drains into z via (psum*invdeg + z); pass B
    (windows 2,3) starts from structural self-edge (diag) matmuls and its
    epilogue fuses deg-normalize + dense term, relu + LN stats, with LN
    normalization done per super-tile so it pipelines.
  - Dense terms computed as fp16 matmuls; lhs loads batched 4 tiles/DMA,
    hw bounce stores batched 8 tiles/DMA.
"""
import os
import numpy as np
from contextlib import ExitStack

N = 100000
C = 256
OUT = 128
NCORES = 8
NPC = N // NCORES          # 12500 nodes per core
P = 128
TILES = (NPC + P - 1) // P  # 98
NPC_PAD = TILES * P         # 12544
ST_TILES = 8                # node-tiles per super-tile
NST = (TILES + ST_TILES - 1) // ST_TILES  # 13
# 4 st-aligned per-core sections -> 4 chunked AllGathers, window == section
SEC_ST = ((0, 3), (3, 6), (6, 9), (9, 13))   # st ranges per section
SROWS = (3072, 3072, 3072, 3328)             # rows per core per section
SB = (0, 3072, 6144, 9216)                   # per-core row offset of section
WSIZE = tuple(NCORES * r for r in SROWS)     # (24576, 24576, 24576, 26624)
WBASE = (0, 24576, 49152, 73728)             # global table row offsets
NWIN = 4
LN_EPS = 1e-5

_COMPILED = {}


# ---------------------------------------------------------------- host side
def _sigmoid(x):
    return 1.0 / (1.0 + np.exp(-np.float64(x)))


def _pos_of(r):
    """Global node id -> message-table row under the four-section layout."""
    c = r // NPC
    o = r % NPC
    k = np.digitize(o, SB[1:])
    kb = np.asarray(WBASE)[k]
    return kb + c * np.asarray(SROWS)[k] + (o - np.asarray(SB)[k])


def _build_layer(src, dst):
    """Vectorized layout builder. Slot stream is ordered (pass, st, w, tile, s)
    so pass A (windows 0,1) is a contiguous prefix. Returns dict with:
       runs: flat list of run dicts (pass, sti, w, n_slots, mm_list,
             slot_base, mm_base) in stream order
       idx:  [NCORES, 128, S/16] int16 window-local gather indices
       sel:  [NCORES, 128, NMM*128] selector blobs
       diag: [NCORES, 128, TILES*128] self-edge multiplicity blobs
    """
    per_core = []
    diag = np.zeros((NCORES, 128, TILES * 128), dtype=np.uint8)
    for c in range(NCORES):
        lo = c * NPC
        m = (src >= lo) & (src < lo + NPC)
        # structural self-edges bypass the gather: counted into a diagonal blob
        selfm = m & (src == dst)
        si = (src[selfm] - lo).astype(np.int64)
        mult = np.bincount(si, minlength=NPC_PAD)
        pp = np.arange(NPC_PAD)
        diag[c, pp & 127, (pp >> 7) * 128 + (pp & 127)] = mult
        m = m & (src != dst)
        s = (src[m] - lo).astype(np.int64)
        d = dst[m].astype(np.int64)
        pos = _pos_of(d)
        w = np.digitize(pos, WBASE[1:])  # window index 0..3
        t = s >> 7
        sti = t >> 3
        wp = w >> 1                      # pass (0: windows 0/1, 1: windows 2/3)
        order = np.lexsort((s, t, w, sti, wp))
        per_core.append((s[order], (pos - np.asarray(WBASE)[w])[order],
                         w[order], t[order], sti[order]))

    cnt = np.zeros((NCORES, NST, NWIN, TILES), dtype=np.int64)
    for c in range(NCORES):
        s, d, w, t, sti = per_core[c]
        np.add.at(cnt[c], (sti, w, t), 1)
    ucnt = cnt.max(axis=0)

    runs = []
    total_slots = 0
    total_mms = 0
    slot_tile_all = []
    seg_start = {}  # (st,w,tile) -> unified slot start position
    for wp in range(2):
        for sti in range(NST):
            for wi in (2 * wp, 2 * wp + 1):
                segs = [(ti, int(ucnt[sti, wi, ti]))
                        for ti in range(sti * ST_TILES, min((sti + 1) * ST_TILES, TILES))
                        if ucnt[sti, wi, ti] > 0]
                maxtpb = int(os.environ.get("K_MAXTPB", "4"))
                slots = []
                tiles_in_blk = set()
                f = 0
                for (ti, n) in segs:
                    if f > 0 and len(tiles_in_blk) >= maxtpb and ti not in tiles_in_blk:
                        slots.extend([-1] * (128 - f))
                        f = 0
                        tiles_in_blk = set()
                    seg_start[(sti, wi, ti)] = total_slots + len(slots)
                    rem = n
                    while rem > 0:
                        take = min(128 - f, rem)
                        slots.extend([ti] * take)
                        f += take
                        rem -= take
                        tiles_in_blk.add(ti)
                        if f == 128:
                            f = 0
                            tiles_in_blk = set()
                if f > 0:
                    slots.extend([-1] * (128 - f))
                n_slots = len(slots)
                nblk = n_slots // 128
                mm_list = []
                for b in range(nblk):
                    blk = slots[b * 128:(b + 1) * 128]
                    touched = []
                    for q in blk:
                        if q >= 0 and q not in touched:
                            touched.append(q)
                    for ti in touched:
                        mm_list.append((b, ti))
                runs.append({"pass": wp, "sti": sti, "w": wi, "n_slots": n_slots,
                             "mm_list": mm_list, "slot_base": total_slots,
                             "mm_base": total_mms})
                slot_tile_all.extend(slots)
                total_slots += n_slots
                total_mms += len(mm_list)

    slot_tile_all = np.asarray(slot_tile_all, dtype=np.int64)

    # per-core: place edges into unified slots
    idx = np.zeros((NCORES, total_slots), dtype=np.int16)
    selcol = np.full((NCORES, total_slots), -1, dtype=np.int64)  # src col (s%128)
    for c in range(NCORES):
        s, d, w, t, sti = per_core[c]
        wp = w >> 1
        key = (((wp * NST + sti) * NWIN) + w) * TILES + t
        change = np.empty(len(key), dtype=bool)
        if len(key):
            change[0] = True
            change[1:] = key[1:] != key[:-1]
        grp_start_idx = np.flatnonzero(change)
        grp_of_edge = np.cumsum(change) - 1
        offset_in_grp = np.arange(len(key)) - grp_start_idx[grp_of_edge]
        base = np.array([seg_start[(int(sti[i]), int(w[i]), int(t[i]))]
                         for i in grp_start_idx], dtype=np.int64)
        slot_pos = base[grp_of_edge] + offset_in_grp
        idx[c, slot_pos] = d.astype(np.int16)
        selcol[c, slot_pos] = s & 127

    # selector blob: [128 partitions(slot within block), total_mms*128]
    sel = np.zeros((NCORES, 128, total_mms * 128), dtype=np.uint8)
    mm_i_global = 0
    for run in runs:
        sb = run["slot_base"]
        for (b, ti) in run["mm_list"]:
            sl0 = sb + b * 128
            tile_match = slot_tile_all[sl0:sl0 + 128] == ti
            for c in range(NCORES):
                cols = selcol[c, sl0:sl0 + 128]
                jj = np.flatnonzero(tile_match & (cols >= 0))
                sel[c, jj, mm_i_global * 128 + cols[jj]] = 1
            mm_i_global += 1
    assert mm_i_global == total_mms

    # wrapped idx arrays: [128, total_slots/16]; slot i -> [i%16 (+16g), i//16]
    assert total_slots % 128 == 0
    idx_w = np.zeros((NCORES, 16, total_slots // 16), dtype=np.int16)
    ar = np.arange(total_slots)
    idx_w[:, ar % 16, ar // 16] = idx
    idx_w = np.tile(idx_w, (1, 8, 1))

    # int32 offsets for the hardware indirect-DMA path: slot i -> [i%128, i//128]
    idx32 = np.zeros((NCORES, 128, total_slots // 128), dtype=np.int32)
    idx32[:, ar % 128, ar // 128] = idx

    return {"runs": runs, "total_slots": total_slots, "total_mms": total_mms,
            "idx": idx_w, "idx32": idx32, "sel": sel, "diag": diag}


def _prep(inputs):
    """All host-side preprocessing -> per-core input maps + static meta."""
    import ml_dtypes
    f16 = np.float16
    x = np.asarray(inputs["x"], np.float32)
    ei1 = np.asarray(inputs["edge_index_r1"])
    ei0 = np.asarray(inputs["edge_index_r0"])

    g1 = np.float32(_sigmoid(inputs["gate1"]))
    g0 = np.float32(_sigmoid(inputs["gate0"]))
    lns1 = np.asarray(inputs["lns1"], np.float32); lnb1 = np.asarray(inputs["lnb1"], np.float32)
    lns0 = np.asarray(inputs["lns0"], np.float32); lnb0 = np.asarray(inputs["lnb0"], np.float32)

    wl1 = np.asarray(inputs["wl1"], np.float32); bl1 = np.asarray(inputs["bl1"], np.float32)
    w01 = np.asarray(inputs["w01"], np.float32); b01 = np.asarray(inputs["b01"], np.float32)
    w11 = np.asarray(inputs["w11"], np.float32); b11 = np.asarray(inputs["b11"], np.float32)
    wl0 = np.asarray(inputs["wl0"], np.float32); bl0 = np.asarray(inputs["bl0"], np.float32)
    w00 = np.asarray(inputs["w00"], np.float32); b00 = np.asarray(inputs["b00"], np.float32)
    w10 = np.asarray(inputs["w10"], np.float32); b10 = np.asarray(inputs["b10"], np.float32)
    Wout = np.asarray(inputs["Wout"], np.float32); bout = np.asarray(inputs["bout"], np.float32)

    # Layer 1 (uses r1 edges, params *1); h = x0 = x
    rhsAB1 = np.concatenate([wl1.T, ((1 - g1) * w01 + g1 * w11).T], axis=1)  # [256,512]
    bias1 = bl1 + (1 - g1) * b01 + g1 * b11                                   # [256]
    # Layer 2 (r0 edges, params *0); h = h1 = h1r*lns1 + lnb1, x0 = x
    wl0T_s = (lns1[:, None] * wl0.T)            # [256,256] for h1r path
    const1 = lnb1 @ wl0.T                        # [256] message const
    w00T_s = (1 - g0) * (lns1[:, None] * w00.T)
    rhsA2 = np.concatenate([wl0T_s, w00T_s], axis=1)  # [256,512]
    w10T_s = g0 * w10.T                          # [256,256] x path
    bias2 = bl0 + (1 - g0) * (b00 + lnb1 @ w00.T) + g0 * b10
    crow2 = np.concatenate([const1, bias2])      # [512]
    # Final: out = h2 @ WoutT + bout, h2 = h2r*lns0 + lnb0
    WoutT_s = lns0[:, None] * Wout.T             # [256,128]
    bout_s = bout + lnb0 @ Wout.T                # [128]

    # degree normalization (bincount over dst of the FULL edge list)
    inv1 = 1.0 / np.clip(np.bincount(ei1[1], minlength=N), 1.0, None).astype(np.float32)
    inv0 = 1.0 / np.clip(np.bincount(ei0[1], minlength=N), 1.0, None).astype(np.float32)

    lay1 = _build_layer(ei1[0].astype(np.int64), ei1[1].astype(np.int64))
    lay2 = _build_layer(ei0[0].astype(np.int64), ei0[1].astype(np.int64))

    fp8 = ml_dtypes.float8_e4m3
    in_maps = []
    for c in range(NCORES):
        lo = c * NPC
        xs = np.zeros((NPC_PAD, C), np.float32)
        xs[:NPC] = x[lo:lo + NPC]
        inv1c = np.zeros(NPC_PAD, np.float32); inv1c[:NPC] = inv1[lo:lo + NPC]
        inv0c = np.zeros(NPC_PAD, np.float32); inv0c[:NPC] = inv0[lo:lo + NPC]
        in_maps.append(dict(
            xT=np.ascontiguousarray(xs.T).astype(f16),
            rhsAB1=rhsAB1.astype(f16), brow1=bias1[None, :].astype(f16),
            rhsA2=rhsA2.astype(f16), rhsY2=w10T_s.astype(f16), crow2=crow2[None, :].astype(f16),
            rhsF=WoutT_s.astype(f16), browF=bout_s[None, :].astype(f16),
            invdeg1=np.ascontiguousarray(inv1c.reshape(TILES, P).T),
            invdeg2=np.ascontiguousarray(inv0c.reshape(TILES, P).T),
            idx1=lay1["idx"][c], idx2=lay2["idx"][c],
            idx321=lay1["idx32"][c], idx322=lay2["idx32"][c],
            sel1=lay1["sel"][c].astype(np.float32).astype(fp8),
            sel2=lay2["sel"][c].astype(np.float32).astype(fp8),
            diag1=lay1["diag"][c].astype(np.float32).astype(fp8),
            diag2=lay2["diag"][c].astype(np.float32).astype(fp8),
        ))
    return in_maps, lay1, lay2


# ---------------------------------------------------------------- device side
def _build_nc(lay1, lay2):
    import concourse.bass as bass
    import concourse.tile as tile
    from concourse import bacc, mybir
    from concourse.masks import make_identity

    f32, f16 = mybir.dt.float32, mybir.dt.float16
    f8, i16 = mybir.dt.float8e4, mybir.dt.int16
    i32 = mybir.dt.int32
    AF = mybir.ActivationFunctionType
    OP = mybir.AluOpType
    use_ind = os.environ.get("K_IND", "0") == "1"

    nc = bacc.Bacc("TRN2", target_bir_lowering=False, debug=False,
                   num_devices=NCORES, num_swdge_queues=4,
                   dynamic_dma_scratch_size=49152)

    S1, S2 = lay1["total_slots"], lay2["total_slots"]
    M1, M2 = lay1["total_mms"], lay2["total_mms"]

    xT_in = nc.dram_tensor("xT", [C, NPC_PAD], f16, kind="ExternalInput").ap()
    rhsAB1_in = nc.dram_tensor("rhsAB1", [C, 512], f16, kind="ExternalInput").ap()
    brow1_in = nc.dram_tensor("brow1", [1, 256], f16, kind="ExternalInput").ap()
    rhsA2_in = nc.dram_tensor("rhsA2", [C, 512], f16, kind="ExternalInput").ap()
    rhsY2_in = nc.dram_tensor("rhsY2", [C, 256], f16, kind="ExternalInput").ap()
    crow2_in = nc.dram_tensor("crow2", [1, 512], f16, kind="ExternalInput").ap()
    rhsF_in = nc.dram_tensor("rhsF", [C, OUT], f16, kind="ExternalInput").ap()
    browF_in = nc.dram_tensor("browF", [1, OUT], f16, kind="ExternalInput").ap()
    invdeg1_in = nc.dram_tensor("invdeg1", [P, TILES], f32, kind="ExternalInput").ap()
    invdeg2_in = nc.dram_tensor("invdeg2", [P, TILES], f32, kind="ExternalInput").ap()
    idx1_in = nc.dram_tensor("idx1", [P, S1 // 16], i16, kind="ExternalInput").ap()
    idx2_in = nc.dram_tensor("idx2", [P, S2 // 16], i16, kind="ExternalInput").ap()
    if use_ind:
        idx321_in = nc.dram_tensor("idx321", [P, S1 // 128], i32, kind="ExternalInput").ap()
        idx322_in = nc.dram_tensor("idx322", [P, S2 // 128], i32, kind="ExternalInput").ap()
    sel1_in = nc.dram_tensor("sel1", [P, M1 * P], f8, kind="ExternalInput").ap()
    sel2_in = nc.dram_tensor("sel2", [P, M2 * P], f8, kind="ExternalInput").ap()
    diag1_in = nc.dram_tensor("diag1", [P, TILES * P], f8, kind="ExternalInput").ap()
    diag2_in = nc.dram_tensor("diag2", [P, TILES * P], f8, kind="ExternalInput").ap()

    out_dram = nc.dram_tensor("out", [NPC_PAD, OUT], f32, kind="ExternalOutput").ap()

    # separate DRAM tensors per table section so each window's gathers depend
    # only on its own chunked AllGather (DRAM dep tracking is per-tensor)
    ags = {(l, k): nc.dram_tensor(f"ag{l}_{k}", [SROWS[k], C], f16)
           for l in (1, 2) for k in range(4)}
    tabs = {(l, k): nc.dram_tensor(f"tab{l}_{k}", [WSIZE[k], C], f16,
                                   addr_space="Shared")
            for l in (1, 2) for k in range(4)}

    # per-(st, pass) sizing for gather/selector staging tiles
    def _st_sizes(lay):
        blk = np.zeros((2, NST), dtype=np.int64)
        mmn = np.zeros((2, NST), dtype=np.int64)
        for run in lay["runs"]:
            blk[run["pass"], run["sti"]] += run["n_slots"] // 128
            mmn[run["pass"], run["sti"]] += len(run["mm_list"])
        return blk, mmn

    blk1, mmn1 = _st_sizes(lay1)
    blk2, mmn2 = _st_sizes(lay2)
    max_blk = int(max(blk1.max(), blk2.max()))
    max_mm = int(max(mmn1.max(), mmn2.max()))

    with tile.TileContext(nc) as tc, ExitStack() as ctx:
        sb = ctx.enter_context(tc.tile_pool(name="sb", bufs=1))
        lhs_pool = ctx.enter_context(tc.tile_pool(name="lhs", bufs=2))
        hwst_pool = ctx.enter_context(tc.tile_pool(name="hwst", bufs=2))
        small = ctx.enter_context(tc.tile_pool(name="small", bufs=2))
        gpool = ctx.enter_context(tc.tile_pool(name="gst", bufs=2))
        spool = ctx.enter_context(tc.tile_pool(name="sel", bufs=2))
        dpool = ctx.enter_context(tc.tile_pool(name="dg", bufs=2))
        epi = ctx.enter_context(tc.tile_pool(name="epi", bufs=4))

        # persistent staging
        z_sb = sb.tile([P, TILES * C], f16)      # dense term, then relu output r
        h_sb = sb.tile([P, TILES * C], f16)      # LN output (h1r, then h2r)
        s1_all = sb.tile([P, TILES], f32)
        s2_all = sb.tile([P, TILES], f32)
        invdeg1_sb = sb.tile([P, TILES], f32)
        invdeg2_sb = sb.tile([P, TILES], f32)
        ident16 = sb.tile([P, P], f16)
        make_identity(nc, ident16[:])
        ones_col = sb.tile([1, P], f16)
        nc.vector.memset(ones_col[:], 1.0)
        nc.sync.dma_start(invdeg1_sb[:], invdeg1_in[:])
        nc.sync.dma_start(invdeg2_sb[:], invdeg2_in[:])

        # weight tiles (persistent)
        rhsAB1_sb = sb.tile([C // 2, 2, 512], f16)
        nc.sync.dma_start(rhsAB1_sb[:], rhsAB1_in[:].rearrange("(b k) n -> k b n", k=128))
        brow1_sb = sb.tile([1, 256], f16)
        nc.sync.dma_start(brow1_sb[:], brow1_in[:])
        rhsA2_sb = sb.tile([C // 2, 2, 512], f16)
        nc.sync.dma_start(rhsA2_sb[:], rhsA2_in[:].rearrange("(b k) n -> k b n", k=128))
        rhsY2_sb = sb.tile([C // 2, 2, 256], f16)
        nc.sync.dma_start(rhsY2_sb[:], rhsY2_in[:].rearrange("(b k) n -> k b n", k=128))
        crow2_sb = sb.tile([1, 512], f16)
        nc.sync.dma_start(crow2_sb[:], crow2_in[:])
        rhsF_sb = sb.tile([C // 2, 2, OUT], f16)
        nc.sync.dma_start(rhsF_sb[:], rhsF_in[:].rearrange("(b k) n -> k b n", k=128))
        browF_sb = sb.tile([1, OUT], f16)
        nc.sync.dma_start(browF_sb[:], browF_in[:])

        if use_ind:
            idx1_sb = sb.tile([P, S1 // 128], i32)
            nc.sync.dma_start(idx1_sb[:], idx321_in[:])
            idx2_sb = sb.tile([P, S2 // 128], i32)
            nc.sync.dma_start(idx2_sb[:], idx322_in[:])
        else:
            idx1_sb = sb.tile([P, S1 // 16], i16)
            nc.sync.dma_start(idx1_sb[:], idx1_in[:])
            idx2_sb = sb.tile([P, S2 // 16], i16)
            nc.sync.dma_start(idx2_sb[:], idx2_in[:])

        def dense_phase(layer):
            """Compute [hw | z] per tile; hw -> sectioned ag bounces, z -> z_sb."""
            with tc.tile_pool(name="abps", bufs=4, space="PSUM") as abps, \
                 tc.tile_pool(name="trps", bufs=4, space="PSUM") as trps, \
                 tc.tile_pool(name="abe", bufs=4) as abe:
                lh4 = None
                stage = None
                for t in range(TILES):
                    if t % 4 == 0:
                        nt = min(4, TILES - t)
                        lh4 = lhs_pool.tile([P, 2, 4 * P], f16, tag="xT4")
                        nc.sync.dma_start(
                            lh4[:, :, 0:nt * P],
                            xT_in[:, t * P:(t + nt) * P].rearrange("(b k) n -> k b n", k=128))
                    j = (t % 4) * P
                    acc = abps.tile([P, 512], f32, space="PSUM")
                    if layer == 1:
                        nc.tensor.matmul(acc[:], lhsT=lh4[:, 0, j:j + P], rhs=rhsAB1_sb[:, 0, :], start=True, stop=False)
                        nc.tensor.matmul(acc[:], lhsT=lh4[:, 1, j:j + P], rhs=rhsAB1_sb[:, 1, :], start=False, stop=False)
                        nc.tensor.matmul(acc[:, 256:512], lhsT=ones_col[:], rhs=brow1_sb[:],
                                         start=False, stop=True, skip_group_check=True)
                    else:
                        # transpose h1r tile -> lhsT fp16
                        hT = abe.tile([P, 2, P], f16, tag="hT")
                        for k in range(2):
                            tp = trps.tile([P, P], f16, space="PSUM", tag="trp")
                            nc.tensor.transpose(tp[:], h_sb[:, t * C + k * P: t * C + (k + 1) * P], ident16[:])
                            nc.vector.tensor_copy(hT[:, k, :], tp[:])
                        nc.tensor.matmul(acc[:], lhsT=hT[:, 0, :], rhs=rhsA2_sb[:, 0, :], start=True, stop=False)
                        nc.tensor.matmul(acc[:], lhsT=hT[:, 1, :], rhs=rhsA2_sb[:, 1, :], start=False, stop=False)
                        nc.tensor.matmul(acc[:, 256:512], lhsT=lh4[:, 0, j:j + P], rhs=rhsY2_sb[:, 0, :],
                                         start=False, stop=False, skip_group_check=True)
                        nc.tensor.matmul(acc[:, 256:512], lhsT=lh4[:, 1, j:j + P], rhs=rhsY2_sb[:, 1, :],
                                         start=False, stop=False, skip_group_check=True)
                        nc.tensor.matmul(acc[:], lhsT=ones_col[:], rhs=crow2_sb[:],
                                         start=False, stop=True, skip_group_check=True)
                    # hw half -> fp16 staging; one bounce DMA per super-tile
                    if t % ST_TILES == 0:
                        stage = hwst_pool.tile([P, ST_TILES * C], f16, tag="hwstage")
                    nc.scalar.activation(stage[:, (t % ST_TILES) * C:(t % ST_TILES + 1) * C],
                                         acc[:, 0:256], AF.Copy)
                    if t % ST_TILES == ST_TILES - 1 or t == TILES - 1:
                        st = t // ST_TILES
                        t0 = st * ST_TILES
                        ntl = t - t0 + 1
                        k = min(st // 3, 3)
                        dst = ags[(layer, k)][t0 * P - SB[k]: t0 * P - SB[k] + ntl * P, :]
                        nc.sync.dma_start(
                            dst.rearrange("(b k) n -> k b n", k=128),
                            stage[:, 0:ntl * C].rearrange("p (b c) -> p b c", c=C))
                        if st == SEC_ST[k][1] - 1:
                            # section complete -> kick its AllGather now
                            nc.gpsimd.collective_compute(
                                "AllGather", mybir.AluOpType.bypass,
                                replica_groups=[list(range(NCORES))],
                                ins=[ags[(layer, k)][:].opt()],
                                outs=[tabs[(layer, k)][:].opt()])
                    # z half -> z_sb fp16
                    nc.vector.tensor_copy(z_sb[:, t * C:(t + 1) * C], acc[:, 256:512])

        def gather_pass(lnum, lay, idx_sb, sel_in_ap, invdeg_sb, diag_in_ap,
                        wp, cps):
            """One gather pass (wp=0: windows 0,1 drain into z; wp=1: windows
            2,3 + diag, epilogue with relu/LN)."""
            runs_by_st = {}
            for run in lay["runs"]:
                if run["pass"] == wp:
                    runs_by_st.setdefault(run["sti"], []).append(run)
            for sti in range(NST):
                st_runs = runs_by_st.get(sti, [])
                st_nblk = sum(r["n_slots"] for r in st_runs) // 128
                st_nmm = sum(len(r["mm_list"]) for r in st_runs)
                st_t0 = sti * ST_TILES
                st_ntiles = min(ST_TILES, TILES - st_t0)
                if st_nblk == 0 and wp == 0:
                    continue
                g_sb = None
                sel_sb = None
                if st_nblk > 0:
                    g_sb = gpool.tile([P, max_blk * C], f16, tag="g")
                    sel_sb = spool.tile([P, max_mm * P], f8, tag="s")
                    mm_b0 = st_runs[0]["mm_base"]
                    nc.sync.dma_start(sel_sb[:, 0:st_nmm * P],
                                      sel_in_ap[:, mm_b0 * P:(mm_b0 + st_nmm) * P])
                    # gathers per run: PREPARE_ONLY + trigger, round-robin queues
                    blk_off = 0
                    for ri, run in enumerate(st_runs):
                        ns = run["n_slots"]
                        if ns == 0:
                            continue
                        wi = run["w"]
                        sb0 = run["slot_base"]
                        nb = ns // 128
                        gout = g_sb[:, blk_off * C:(blk_off + nb) * C].rearrange("p (b c) -> p b c", c=C)
                        if use_ind:
                            nc.gpsimd.indirect_dma_start(
                                out=gout, out_offset=None,
                                in_=tabs[(lnum, wi)][:],
                                in_offset=bass.IndirectOffsetOnAxis(
                                    ap=idx_sb[:, sb0 // 128:(sb0 + ns) // 128],
                                    axis=0),
                            )
                        else:
                            gidx = idx_sb[:, sb0 // 16:(sb0 + ns) // 16]
                            nc.gpsimd.dma_gather(
                                out_ap=gout, in_ap=tabs[(lnum, wi)][:],
                                idxs_ap=gidx, num_idxs=ns, num_idxs_reg=ns,
                                elem_size=C,
                            )
                        blk_off += nb
                # psum tiles: 4 banks x [128,512] = 8 node-tiles
                accs = [cps.tile([P, 512], f32, space="PSUM", tag="agg", name=f"agg{wp}_{sti}_{_i}")
                        for _i in range((st_ntiles + 1) // 2)]
                # mm bookkeeping
                mm_seq = []
                blk_off = 0
                for run in st_runs:
                    for (b, ti) in run["mm_list"]:
                        mm_seq.append((blk_off + b, ti))
                    blk_off += run["n_slots"] // 128
                last = {}
                first_of_pair = {}
                for i, (b, ti) in enumerate(mm_seq):
                    last[ti] = i
                    pr = (ti - st_t0) // 2
                    if pr not in first_of_pair:
                        first_of_pair[pr] = i
                if wp == 1:
                    # structural self-edges clear every bank: diag first
                    dg = dpool.tile([P, ST_TILES * P], f8, tag="dg")
                    nc.sync.dma_start(dg[:, 0:st_ntiles * P],
                                      diag_in_ap[:, st_t0 * P:(st_t0 + st_ntiles) * P])
                    hwl = dpool.tile([P, ST_TILES * C], f16, tag="hwl")
                    k = min(sti // 3, 3)
                    src = ags[(lnum, k)][st_t0 * P - SB[k]: st_t0 * P - SB[k] + st_ntiles * P, :]
                    nc.sync.dma_start(hwl[:, 0:st_ntiles * C].rearrange("p (b c) -> p b c", c=C),
                                      src.rearrange("(b k) n -> k b n", k=128))
                    for tl in range(st_ntiles):
                        t = st_t0 + tl
                        reg = accs[tl // 2][:, (tl % 2) * 256:(tl % 2) * 256 + 256]
                        nc.tensor.matmul(reg, lhsT=dg[:, tl * P:(tl + 1) * P],
                                         rhs=hwl[:, tl * C:(tl + 1) * C],
                                         start=(tl % 2 == 0), stop=(last.get(t) is None),
                                         skip_group_check=True)
                for i, (b, ti) in enumerate(mm_seq):
                    tl = ti - st_t0
                    reg = accs[tl // 2][:, (tl % 2) * 256:(tl % 2) * 256 + 256]
                    nc.tensor.matmul(
                        reg, lhsT=sel_sb[:, i * P:(i + 1) * P],
                        rhs=g_sb[:, b * C:(b + 1) * C],
                        start=(wp == 0 and first_of_pair[(tl) // 2] == i),
                        stop=(last[ti] == i),
                        skip_group_check=True,
                    )
                if wp == 0:
                    # drain pass A: z += psum * invdeg  (only tiles with mms)
                    for tl in range(st_ntiles):
                        t = st_t0 + tl
                        if t not in last:
                            continue
                        reg = accs[tl // 2][:, (tl % 2) * 256:(tl % 2) * 256 + 256]
                        nc.vector.scalar_tensor_tensor(
                            out=z_sb[:, t * C:(t + 1) * C], in0=reg,
                            scalar=invdeg_sb[:, t:t + 1],
                            in1=z_sb[:, t * C:(t + 1) * C], op0=OP.mult, op1=OP.add)
                else:
                    # epilogue per node-tile + per-st LN
                    for tl in range(st_ntiles):
                        t = st_t0 + tl
                        reg = accs[tl // 2][:, (tl % 2) * 256:(tl % 2) * 256 + 256]
                        tmp = epi.tile([P, C], f32, tag="etmp")
                        nc.vector.scalar_tensor_tensor(
                            out=tmp[:], in0=reg, scalar=invdeg_sb[:, t:t + 1],
                            in1=z_sb[:, t * C:(t + 1) * C], op0=OP.mult, op1=OP.add)
                        nc.scalar.activation(z_sb[:, t * C:(t + 1) * C], tmp[:], AF.Relu,
                                             accum_out=s1_all[:, t:t + 1])
                        sq = epi.tile([P, C], f16, tag="esq")
                        nc.scalar.activation(sq[:], z_sb[:, t * C:(t + 1) * C], AF.Square,
                                             accum_out=s2_all[:, t:t + 1])
                    # per-st LN stats
                    sl = slice(st_t0, st_t0 + st_ntiles)
                    mu = small.tile([P, ST_TILES], f32, tag="mu")
                    var = small.tile([P, ST_TILES], f32, tag="var")
                    std = small.tile([P, ST_TILES], f32, tag="std")
                    rstd = small.tile([P, ST_TILES], f32, tag="rstd")
                    nn = st_ntiles
                    nc.vector.tensor_scalar(out=mu[:, 0:nn], in0=s1_all[:, sl], scalar1=1.0 / C,
                                            scalar2=None, op0=OP.mult)
                    nc.vector.tensor_tensor(out=var[:, 0:nn], in0=mu[:, 0:nn], in1=mu[:, 0:nn], op=OP.mult)
                    nc.vector.scalar_tensor_tensor(out=var[:, 0:nn], in0=s2_all[:, sl], scalar=1.0 / C,
                                                   in1=var[:, 0:nn], op0=OP.mult, op1=OP.subtract)
                    nc.vector.tensor_scalar(out=var[:, 0:nn], in0=var[:, 0:nn], scalar1=float(LN_EPS),
                                            scalar2=None, op0=OP.add)
                    nc.scalar.activation(std[:, 0:nn], var[:, 0:nn], AF.Sqrt)
                    nc.vector.reciprocal(rstd[:, 0:nn], std[:, 0:nn])
                    for tl in range(st_ntiles):
                        t = st_t0 + tl
                        nc.vector.tensor_scalar(
                            out=h_sb[:, t * C:(t + 1) * C], in0=z_sb[:, t * C:(t + 1) * C],
                            scalar1=mu[:, tl:tl + 1], scalar2=rstd[:, tl:tl + 1],
                            op0=OP.subtract, op1=OP.mult)

        def layer(lnum, lay, idx_sb, sel_in, invdeg_sb, diag_in):
            dense_phase(lnum)  # kicks the 4 chunked AllGathers as sections finish
            with tc.tile_pool(name=f"cpsA{lnum}", bufs=8, space="PSUM") as cpsA:
                gather_pass(lnum, lay, idx_sb, sel_in, invdeg_sb, diag_in, 0, cpsA)
            with tc.tile_pool(name=f"cpsB{lnum}", bufs=8, space="PSUM") as cpsB:
                gather_pass(lnum, lay, idx_sb, sel_in, invdeg_sb, diag_in, 1, cpsB)

        # ---------------- layer 1
        layer(1, lay1, idx1_sb, sel1_in, invdeg1_sb, diag1_in)
        # ---------------- layer 2
        layer(2, lay2, idx2_sb, sel2_in, invdeg2_sb, diag2_in)
        # ---------------- final projection
        with tc.tile_pool(name="fps", bufs=4, space="PSUM") as fps, \
             tc.tile_pool(name="ftr", bufs=4, space="PSUM") as ftr, \
             tc.tile_pool(name="fe", bufs=4) as fe:
            for t in range(TILES):
                hT = fe.tile([P, 2, P], f16, tag="fhT")
                for k in range(2):
                    tp = ftr.tile([P, P], f16, space="PSUM", tag="ftp")
                    nc.tensor.transpose(tp[:], h_sb[:, t * C + k * P: t * C + (k + 1) * P], ident16[:])
                    nc.vector.tensor_copy(hT[:, k, :], tp[:])
                acc = fps.tile([P, OUT], f32, space="PSUM")
                nc.tensor.matmul(acc[:], lhsT=hT[:, 0, :], rhs=rhsF_sb[:, 0, :], start=True, stop=False)
                nc.tensor.matmul(acc[:], lhsT=hT[:, 1, :], rhs=rhsF_sb[:, 1, :], start=False, stop=False)
                nc.tensor.matmul(acc[:], lhsT=ones_col[:], rhs=browF_sb[:],
                                 start=False, stop=True, skip_group_check=True)
                o_sb = fe.tile([P, OUT], f32, tag="fo")
                nc.vector.tensor_copy(o_sb[:], acc[:])
                nc.sync.dma_start(out_dram[t * P:(t + 1) * P, :], o_sb[:])

    nc.compile()
    return nc


# ---------------------------------------------------------------- entry point
def kernel(**inputs):
    from concourse.bass_utils import run_bass_kernel_spmd

    in_maps, lay1, lay2 = _prep(inputs)
    key = "nc"
    if key not in _COMPILED:
        _COMPILED[key] = _build_nc(lay1, lay2)
    nc = _COMPILED[key]
    res = run_bass_kernel_spmd(nc, in_maps, core_ids=list(range(NCORES)))
    _COMPILED["last_res"] = res
    out = np.concatenate([res.results[c]["out"][:NPC] for c in range(NCORES)], axis=0)
    return out.astype(np.float32)
